# revision 11
# baseline (speedup 1.0000x reference)
"""Trainium2 Bass kernel for the BDH-style weight-tied transformer.

Contract: kernel(**inputs) takes FULL unsharded numpy inputs (idx, wte,
encoder, decoder_x, decoder_y, readout) and returns the FULL (B, T, V)
logits, running the model on 8 NeuronCores via run_bass_kernel_spmd.

Sharding: core c -> (b = c // 4, h = c % 4).  Group {0..3} handles batch 0,
{4..7} batch 1.  Within a group: tensor-parallel over heads with
AllGather + local-sum for (a) the head-summed attention matrix and (b) the
y @ encoder projection.  LayerNorm is scale-invariant, so summing heads
(instead of averaging) is exact.  Readout is vocab-split 4 ways per group.

The neuron axis of each head is permuted host-side so RoPE pair partners
(2k, 2k+1) live at the same partition of sibling 128-chunks ("even" chunk
2c / "odd" chunk 2c+1).  The rotation then needs no cross-partition data
movement.  The 1/sqrt(d) attention scale is folded into the cos/sin tables
(d**-0.25 on each factor of the Gram matrix).
"""

import sys

for _p in ("/opt/trn_rl_repo", "/opt/pypackages"):
    if _p not in sys.path:
        sys.path.append(_p)

import ml_dtypes
import numpy as np

import concourse.bass as bass
import concourse.mybir as mybir
import concourse.tile as tile
from concourse import bacc
from concourse.bass_utils import run_bass_kernel_spmd

F32 = mybir.dt.float32
BF16 = mybir.dt.bfloat16
I32 = mybir.dt.int32
AX = mybir.AxisListType
ALU = mybir.AluOpType
ACT = mybir.ActivationFunctionType

# Model dims (hardcoded per problem spec)
B, T, D, H, N, V = 2, 256, 256, 4, 32768, 32000
n_head = N // H            # 8192 neurons per head (one core's slice)
P = 128
NCH = n_head // P          # 64 chunks of 128 neurons
NPAIR = NCH // 2           # 32 pair-chunks
L_LAYERS = 6
LN_EPS = 1e-5
ROPE_BASE = 10000.0
VSLICE = V // 4            # 8000 vocab columns per core
VCH = 500                  # vocab chunk (PSUM bank holds 512 f32)
NVCH = VSLICE // VCH       # 16
GX = 4                     # n-chunks per streamed weight group
S4 = float(n_head) ** -0.25


def _ln_pair(nc, pools, srcs, out, out_dtype_note=""):
    """LayerNorm over the free dim (D=256) of two [128, 256] f32 tiles.

    srcs: list of 2 APs (SBUF or PSUM, f32).  out: [128, 2, 256] tile.
    """
    psmall, pcent, psq = pools["small"], pools["cent"], pools["sq"]
    for i, src in enumerate(srcs):
        ssum = psmall.tile([P, 1], F32, name=f"ln_sum{i}", tag="lnstat")
        nc.vector.tensor_reduce(ssum, src, axis=AX.X, op=ALU.add)
        negmean = psmall.tile([P, 1], F32, name=f"ln_negmean{i}", tag="lnstat")
        nc.vector.tensor_scalar_mul(negmean, ssum, -1.0 / D)
        cent = pcent.tile([P, T], F32, name=f"ln_cent{i}", tag="lncent")
        nc.vector.tensor_scalar_add(cent, src, negmean)
        sq = psq.tile([P, T], F32, name=f"ln_sq{i}", tag="lnsq")
        ss2 = psmall.tile([P, 1], F32, name=f"ln_ss2_{i}", tag="lnstat")
        nc.scalar.activation(sq, cent, ACT.Square, accum_out=ss2)
        std = psmall.tile([P, 1], F32, name=f"ln_std{i}", tag="lnstat")
        nc.scalar.activation(std, ss2, ACT.Sqrt, bias=pools["eps"][:, :1],
                             scale=1.0 / D)
        rinv = psmall.tile([P, 1], F32, name=f"ln_rinv{i}", tag="lnstat")
        nc.vector.reciprocal(rinv, std)
        nc.vector.tensor_scalar_mul(out[:, i, :], cent, rinv)


def _transpose4(nc, pools, src, dst, ident):
    """dst[:, k, 128*i:128*(i+1)] = src[:, i, 128*k:128*(k+1)].T  (bf16).

    src, dst: [128, 2, 256] bf16.  Four PE transposes + ACT copies.
    """
    pwork = pools["ps_work"]
    for i in range(2):
        for k in range(2):
            tp = pwork.tile([P, P], BF16, name=f"tp_{i}_{k}", tag="work")
            nc.tensor.transpose(tp, src[:, i, P * k:P * (k + 1)], ident)
            nc.scalar.copy(dst[:, k, P * i:P * (i + 1)], tp)


def build_nc(num_cores=8):
    nc = bacc.Bacc(
        "TRN2", target_bir_lowering=False, debug=False, num_devices=num_cores
    )

    # ---- DRAM I/O (per-core data supplied via in_maps) ----
    wte_d = nc.dram_tensor("wte", [V, D], F32, kind="ExternalInput").ap()
    idx_d = nc.dram_tensor("idx2", [2, P], I32, kind="ExternalInput").ap()
    wx_d = nc.dram_tensor("wx", [D, n_head], BF16, kind="ExternalInput").ap()
    wy_d = nc.dram_tensor("wy", [D, n_head], BF16, kind="ExternalInput").ap()
    enc_d = nc.dram_tensor("enc", [n_head, D], BF16, kind="ExternalInput").ap()
    ro_d = nc.dram_tensor("ro", [D, VSLICE], BF16, kind="ExternalInput").ap()
    cs_d = nc.dram_tensor("cs", [P, NPAIR, 2 * T], BF16, kind="ExternalInput").ap()
    masks_d = nc.dram_tensor("masks", [P, 2, T], BF16, kind="ExternalInput").ap()
    ident_d = nc.dram_tensor("ident", [P, P], BF16, kind="ExternalInput").ap()
    out_d = nc.dram_tensor("out", [T, VSLICE], F32, kind="ExternalOutput").ap()

    groups = [[0, 1, 2, 3], [4, 5, 6, 7]]

    with tile.TileContext(nc) as tc:
        with (
            tc.tile_pool(name="pers", bufs=1) as pers,
            tc.tile_pool(name="pv", bufs=2) as pv,
            tc.tile_pool(name="pbig", bufs=2) as pbig,
            tc.tile_pool(name="pwx", bufs=2) as pwx,
            tc.tile_pool(name="pwy", bufs=2) as pwy,
            tc.tile_pool(name="pro", bufs=3) as pro,
            tc.tile_pool(name="pxr", bufs=6) as pxr,
            tc.tile_pool(name="py", bufs=6) as py,
            tc.tile_pool(name="psmall", bufs=12) as psmall,
            tc.tile_pool(name="pcent", bufs=4) as pcent,
            tc.tile_pool(name="psq", bufs=2) as psq,
            tc.tile_pool(name="pexp", bufs=8) as pexp,
            tc.tile_pool(name="ps_work", bufs=4, space="PSUM") as ps_work,
            tc.tile_pool(name="ps_accum", bufs=2, space="PSUM") as ps_accum,
            tc.tile_pool(name="dram", bufs=2, space="DRAM") as dram,
        ):
            pools = {
                "small": psmall,
                "cent": pcent,
                "sq": psq,
                "ps_work": ps_work,
            }

            # ---- persistent SBUF tensors ----
            eps_sb = pers.tile([P, 1], F32, name="eps_sb", tag="eps")
            nc.vector.memset(eps_sb, LN_EPS)
            pools["eps"] = eps_sb
            cs_sb = pers.tile([P, NPAIR, 2 * T], BF16, name="cs_sb", tag="cs")
            masks_sb = pers.tile([P, 2, T], BF16, name="masks_sb", tag="masks")
            ident_sb = pers.tile([P, P], BF16, name="ident_sb", tag="ident")
            enc_sb = pers.tile([P, NCH, T], BF16, name="enc_sb", tag="enc")
            x_sb = pers.tile([P, NCH, T], BF16, name="x_sb", tag="x")

            for g in range(4):
                nc.sync.dma_start(
                    cs_sb[:, 8 * g:8 * (g + 1), :], cs_d[:, 8 * g:8 * (g + 1), :]
                )
            nc.sync.dma_start(masks_sb[:], masks_d[:])
            nc.sync.dma_start(ident_sb[:], ident_d[:])
            enc_r = enc_d.rearrange("(c p) d -> p c d", p=P)
            for g in range(8):
                nc.sync.dma_start(
                    enc_sb[:, 8 * g:8 * (g + 1), :], enc_r[:, 8 * g:8 * (g + 1), :]
                )

            # ---- embedding gather + first LN ----
            vraw = pbig.tile([P, 2, T], F32, name="vraw", tag="vraw")
            for i in range(2):
                idx_sb = psmall.tile([P, 1], I32, name=f"idx_sb{i}", tag="idx")
                nc.sync.dma_start(idx_sb, idx_d[i, :].rearrange("(p o) -> p o", o=1))
                nc.gpsimd.indirect_dma_start(
                    out=vraw[:, i, :],
                    out_offset=None,
                    in_=wte_d[:],
                    in_offset=bass.IndirectOffsetOnAxis(ap=idx_sb[:, :1], axis=0),
                )
            v = pv.tile([P, 2, T], F32, name="v_l0", tag="v")
            _ln_pair(nc, pools, [vraw[:, 0, :], vraw[:, 1, :]], v)

            for layer in range(L_LAYERS):
                # ---- v_bf (natural, bf16) and vT (transposed, bf16) ----
                v_bf = pbig.tile([P, 2, T], BF16, name=f"vbf_{layer}", tag="vbf")
                for i in range(2):
                    nc.vector.tensor_copy(v_bf[:, i, :], v[:, i, :])
                vT = pbig.tile([P, 2, T], BF16, name=f"vT_{layer}", tag="vT")
                _transpose4(nc, pools, v_bf, vT, ident_sb)

                # ---- x phase: x = relu(v @ Wx), rope, scores (Gram) ----
                sc0 = ps_accum.tile([P, T], F32, name=f"sc0_{layer}", tag="acc0")
                sc1 = ps_accum.tile([P, T], F32, name=f"sc1_{layer}", tag="acc1")
                scores = [sc0, sc1]
                for grp in range(NPAIR // 2):  # 16 groups of 2 pair-chunks
                    ch0 = 4 * grp  # first of 4 n-chunks in this group
                    if ch0 % GX == 0:
                        wxg = pwx.tile([P, 2, GX * P], BF16,
                                       name=f"wxg_{layer}_{ch0}", tag="wx")
                        for dk in range(2):
                            nc.sync.dma_start(
                                wxg[:, dk, :],
                                wx_d[P * dk:P * (dk + 1),
                                     P * ch0:P * (ch0 + GX)],
                            )
                    for ch in range(ch0, ch0 + 4):
                        co = P * (ch % GX)
                        x_pre = ps_work.tile([P, T], F32,
                                             name=f"xpre_{layer}_{ch}", tag="work")
                        for dk in range(2):
                            nc.tensor.matmul(
                                x_pre,
                                lhsT=wxg[:, dk, co:co + P],
                                rhs=vT[:, dk, :],
                                start=(dk == 0),
                                stop=(dk == 1),
                            )
                        nc.scalar.activation(x_sb[:, ch, :], x_pre, ACT.Relu)
                    # rope over the 2 pair-chunks (even chunks ch0, ch0+2;
                    # odd chunks ch0+1, ch0+3), batched FD=512
                    xe = x_sb[:, ch0:ch0 + 4:2, :]
                    xo = x_sb[:, ch0 + 1:ch0 + 4:2, :]
                    cvw = cs_sb[:, 2 * grp:2 * grp + 2, 0:T]
                    svw = cs_sb[:, 2 * grp:2 * grp + 2, T:2 * T]
                    m_ec = pxr.tile([P, 2, T], BF16, name=f"mec_{layer}_{grp}", tag="xr")
                    m_os = pxr.tile([P, 2, T], BF16, name=f"mos_{layer}_{grp}", tag="xr")
                    m_oc = pxr.tile([P, 2, T], BF16, name=f"moc_{layer}_{grp}", tag="xr")
                    m_es = pxr.tile([P, 2, T], BF16, name=f"mes_{layer}_{grp}", tag="xr")
                    xr_e = pxr.tile([P, 2, T], BF16, name=f"xre_{layer}_{grp}", tag="xr")
                    xr_o = pxr.tile([P, 2, T], BF16, name=f"xro_{layer}_{grp}", tag="xr")
                    nc.vector.tensor_mul(m_ec, xe, cvw)
                    nc.vector.tensor_mul(m_os, xo, svw)
                    nc.vector.tensor_sub(xr_e, m_ec, m_os)
                    nc.vector.tensor_mul(m_oc, xo, cvw)
                    nc.vector.tensor_mul(m_es, xe, svw)
                    nc.vector.tensor_add(xr_o, m_oc, m_es)
                    for q in range(2):  # pair-chunk within group
                        for xr in (xr_e, xr_o):
                            chv = ch0 + 2 * q + (0 if xr is xr_e else 1)
                            for i in range(2):
                                nc.tensor.matmul(
                                    scores[i],
                                    lhsT=xr[:, q, P * i:P * (i + 1)],
                                    rhs=xr[:, q, :],
                                    start=(chv == 0),
                                    stop=(chv == NCH - 1),
                                )

                # ---- softmax (causal, per-head normalized) ----
                attn = pexp.tile([P, 2, T], BF16, name=f"attn_{layer}", tag="attn", bufs=2)
                for i in range(2):
                    mx = psmall.tile([P, 1], F32, name=f"mx_{i}", tag="lnstat")
                    nc.vector.tensor_reduce(mx, scores[i], axis=AX.X, op=ALU.max)
                    negmx = psmall.tile([P, 1], F32, name=f"negmx_{i}", tag="lnstat")
                    nc.vector.tensor_scalar_mul(negmx, mx, -1.0)
                    ex = pexp.tile([P, T], BF16, name=f"ex_{layer}_{i}", tag="ex", bufs=2)
                    nc.scalar.activation(ex, scores[i], ACT.Exp, bias=negmx)
                    nc.vector.tensor_mul(ex, ex, masks_sb[:, i, :])
                    rs = psmall.tile([P, 1], F32, name=f"rs_{i}", tag="lnstat")
                    nc.vector.tensor_reduce(rs, ex, axis=AX.X, op=ALU.add)
                    rcp = psmall.tile([P, 1], F32, name=f"rcp_{i}", tag="lnstat")
                    nc.vector.reciprocal(rcp, rs)
                    nc.vector.tensor_scalar_mul(attn[:, i, :], ex, rcp)

                # ---- AllGather attn over the 4-core group; sum heads ----
                attn_bnc = dram.tile([2 * P, T], BF16,
                                     name=f"attn_bnc_{layer}", tag="attn_in")
                for i in range(2):
                    nc.gpsimd.dma_start(attn_bnc[P * i:P * (i + 1), :], attn[:, i, :])
                attn_gth = dram.tile([8 * P, T], BF16, name=f"attn_gth_{layer}",
                                     tag="attn_out")
                nc.gpsimd.collective_compute(
                    "AllGather", ALU.bypass, replica_groups=groups,
                    ins=[attn_bnc.opt()], outs=[attn_gth.opt()],
                )
                asum = pexp.tile([P, 2, T], BF16, name=f"asum_{layer}", tag="asum", bufs=2)
                for i in range(2):
                    g_t = []
                    for g in range(4):
                        gt = pexp.tile([P, T], BF16,
                                       name=f"ag_{layer}_{i}_{g}", tag="ag", bufs=8)
                        nc.sync.dma_start(gt, attn_gth[2 * P * g + P * i:
                                                       2 * P * g + P * (i + 1), :])
                        g_t.append(gt)
                    s01 = pexp.tile([P, T], BF16, name=f"s01_{i}", tag="ag", bufs=8)
                    s23 = pexp.tile([P, T], BF16, name=f"s23_{i}", tag="ag", bufs=8)
                    nc.vector.tensor_add(s01, g_t[0], g_t[1])
                    nc.vector.tensor_add(s23, g_t[2], g_t[3])
                    nc.vector.tensor_add(asum[:, i, :], s01, s23)

                # ---- transpose summed attn; a = attnT.T @ v; LN(a) ----
                attnT = pexp.tile([P, 2, T], BF16, name=f"attnT_{layer}", tag="attnT", bufs=2)
                _transpose4(nc, pools, asum, attnT, ident_sb)
                a_ps = []
                for i in range(2):
                    ap_i = ps_work.tile([P, T], F32, name=f"aps_{layer}_{i}",
                                        tag="work")
                    for j in range(2):
                        nc.tensor.matmul(
                            ap_i,
                            lhsT=attnT[:, j, P * i:P * (i + 1)],
                            rhs=v_bf[:, j, :],
                            start=(j == 0),
                            stop=(j == 1),
                        )
                    a_ps.append(ap_i)
                lnA = pbig.tile([P, 2, T], BF16, name=f"lnA_{layer}", tag="lnA")
                _ln_pair(nc, pools, a_ps, lnA)
                lnAT = pbig.tile([P, 2, T], BF16, name=f"lnAT_{layer}", tag="lnAT")
                _transpose4(nc, pools, lnA, lnAT, ident_sb)

                # ---- y phase: y = relu(lnA @ Wy) * x;  yenc = y @ enc ----
                ye0 = ps_accum.tile([P, T], F32, name=f"ye0_{layer}", tag="acc0")
                ye1 = ps_accum.tile([P, T], F32, name=f"ye1_{layer}", tag="acc1")
                yenc = [ye0, ye1]
                for ch in range(NCH):
                    if ch % GX == 0:
                        wyg = pwy.tile([P, 2, GX * P], BF16,
                                       name=f"wyg_{layer}_{ch}", tag="wy")
                        for dk in range(2):
                            nc.sync.dma_start(
                                wyg[:, dk, :],
                                wy_d[P * dk:P * (dk + 1), P * ch:P * (ch + GX)],
                            )
                    co = P * (ch % GX)
                    y_pre = ps_work.tile([P, T], F32, name=f"ypre_{layer}_{ch}",
                                         tag="work")
                    for dk in range(2):
                        nc.tensor.matmul(
                            y_pre,
                            lhsT=wyg[:, dk, co:co + P],
                            rhs=lnAT[:, dk, :],
                            start=(dk == 0),
                            stop=(dk == 1),
                        )
                    yr = py.tile([P, T], BF16, name=f"yr_{layer}_{ch}", tag="y")
                    nc.scalar.activation(yr, y_pre, ACT.Relu)
                    yt = py.tile([P, T], BF16, name=f"yt_{layer}_{ch}", tag="y")
                    nc.vector.tensor_mul(yt, yr, x_sb[:, ch, :])
                    for i in range(2):
                        nc.tensor.matmul(
                            yenc[i],
                            lhsT=yt[:, P * i:P * (i + 1)],
                            rhs=enc_sb[:, ch, :],
                            start=(ch == 0),
                            stop=(ch == NCH - 1),
                        )

                # ---- AllGather yenc partials (f32) + sum + LNs + residual ----
                ye_bnc = dram.tile([2 * P, T], F32, name=f"ye_bnc_{layer}",
                                   tag="ye_in")
                for i in range(2):
                    ye_sb = pexp.tile([P, T], F32, name=f"ye_sb_{layer}_{i}",
                                      tag="yg", bufs=8)
                    nc.scalar.copy(ye_sb, yenc[i])
                    nc.sync.dma_start(ye_bnc[P * i:P * (i + 1), :], ye_sb)
                ye_gth = dram.tile([8 * P, T], F32, name=f"ye_gth_{layer}",
                                   tag="ye_out")
                nc.gpsimd.collective_compute(
                    "AllGather", ALU.bypass, replica_groups=groups,
                    ins=[ye_bnc.opt()], outs=[ye_gth.opt()],
                )
                ysum = pbig.tile([P, 2, T], F32, name=f"ysum_{layer}", tag="ysum")
                for i in range(2):
                    g_t = []
                    for g in range(4):
                        gt = pexp.tile([P, T], F32, name=f"yg_{layer}_{i}_{g}",
                                       tag="yg", bufs=8)
                        nc.sync.dma_start(gt, ye_gth[2 * P * g + P * i:
                                                     2 * P * g + P * (i + 1), :])
                        g_t.append(gt)
                    s01 = pexp.tile([P, T], F32, name=f"ys01_{i}", tag="yg", bufs=8)
                    s23 = pexp.tile([P, T], F32, name=f"ys23_{i}", tag="yg", bufs=8)
                    nc.vector.tensor_add(s01, g_t[0], g_t[1])
                    nc.vector.tensor_add(s23, g_t[2], g_t[3])
                    nc.vector.tensor_add(ysum[:, i, :], s01, s23)
                lnY = pbig.tile([P, 2, T], F32, name=f"lnY_{layer}", tag="lnY")
                _ln_pair(nc, pools, [ysum[:, 0, :], ysum[:, 1, :]], lnY)
                vres = pbig.tile([P, 2, T], F32, name=f"vres_{layer}", tag="vres")
                for i in range(2):
                    nc.vector.tensor_add(vres[:, i, :], v[:, i, :], lnY[:, i, :])
                v = pv.tile([P, 2, T], F32, name=f"v_l{layer + 1}", tag="v")
                _ln_pair(nc, pools, [vres[:, 0, :], vres[:, 1, :]], v)

            # ---- readout: out = v @ ro  (vocab slice) ----
            v_bf = pbig.tile([P, 2, T], BF16, name="vbf_ro", tag="vbf")
            for i in range(2):
                nc.vector.tensor_copy(v_bf[:, i, :], v[:, i, :])
            vT = pbig.tile([P, 2, T], BF16, name="vT_ro", tag="vT")
            _transpose4(nc, pools, v_bf, vT, ident_sb)
            for c in range(NVCH):
                rog = pro.tile([P, 2, VCH], BF16, name=f"rog_{c}", tag="ro")
                for dk in range(2):
                    nc.sync.dma_start(
                        rog[:, dk, :],
                        ro_d[P * dk:P * (dk + 1), VCH * c:VCH * (c + 1)],
                    )
                for i in range(2):
                    lg = ps_work.tile([P, VCH], F32, name=f"lg_{c}_{i}", tag="work")
                    for dk in range(2):
                        nc.tensor.matmul(
                            lg,
                            lhsT=vT[:, dk, P * i:P * (i + 1)],
                            rhs=rog[:, dk, :],
                            start=(dk == 0),
                            stop=(dk == 1),
                        )
                    lg_sb = py.tile([P, VCH], F32, name=f"lg_sb_{c}_{i}",
                                    tag="lgsb", bufs=4)
                    nc.vector.tensor_copy(lg_sb, lg)
                    nc.sync.dma_start(
                        out_d[P * i:P * (i + 1), VCH * c:VCH * (c + 1)], lg_sb
                    )

    nc.compile()
    return nc


# ------------------------- host-side preparation -------------------------

def _pair_perm():
    """perm[new] = old index within a head, de-interleaving rope pairs."""
    perm = np.zeros(n_head, dtype=np.int64)
    for c in range(NPAIR):
        k = np.arange(P) + c * P          # pair indices in this pair-chunk
        perm[(2 * c) * P + np.arange(P)] = 2 * k
        perm[(2 * c + 1) * P + np.arange(P)] = 2 * k + 1
    return perm


def _rope_tables():
    """cs[p, c, 0:T] = cos, cs[p, c, T:2T] = sin, scaled by d**-0.25."""
    inv_freq = 1.0 / (
        ROPE_BASE ** (np.arange(0, n_head, 2, dtype=np.float32) / n_head)
    )  # (4096,) f32, matching reference arithmetic
    t = np.arange(T, dtype=np.float32)
    freqs = t[:, None] * inv_freq[None, :]         # (T, 4096) f32
    cos = np.cos(freqs) * S4                       # (T, 4096)
    sin = np.sin(freqs) * S4
    cs = np.zeros((P, NPAIR, 2 * T), dtype=np.float32)
    for c in range(NPAIR):
        k = c * P + np.arange(P)                   # (128,) pair indices
        cs[:, c, 0:T] = cos[:, k].T
        cs[:, c, T:2 * T] = sin[:, k].T
    return cs.astype(ml_dtypes.bfloat16)


def _masks():
    m = np.zeros((P, 2, T), dtype=np.float32)
    for i in range(2):
        t = i * P + np.arange(P)[:, None]          # (128,1) row positions
        s = np.arange(T)[None, :]
        m[:, i, :] = (s <= t).astype(np.float32)
    return m.astype(ml_dtypes.bfloat16)


_CACHE = {}


def kernel(idx, wte, encoder, decoder_x, decoder_y, readout):
    if "nc" not in _CACHE:
        _CACHE["nc"] = build_nc()
    nc = _CACHE["nc"]
    in_maps = prepare_in_maps(idx, wte, encoder, decoder_x, decoder_y, readout)
    res = run_bass_kernel_spmd(nc, in_maps, core_ids=list(range(8)))
    return assemble_output([res.results[c]["out"] for c in range(8)])


def assemble_output(outs):
    out = np.empty((B, T, V), dtype=np.float32)
    for c in range(8):
        b, h = c // 4, c % 4
        out[b, :, h * VSLICE:(h + 1) * VSLICE] = outs[c]
    return out


def prepare_in_maps(idx, wte, encoder, decoder_x, decoder_y, readout):
    idx = np.asarray(idx)
    wte = np.ascontiguousarray(np.asarray(wte, dtype=np.float32))
    encoder = np.asarray(encoder, dtype=np.float32)
    decoder_x = np.asarray(decoder_x, dtype=np.float32)
    decoder_y = np.asarray(decoder_y, dtype=np.float32)
    readout = np.asarray(readout, dtype=np.float32)

    perm = _pair_perm()
    cs = _rope_tables()
    masks = _masks()
    ident = np.eye(P, dtype=np.float32).astype(ml_dtypes.bfloat16)
    bf = ml_dtypes.bfloat16

    wx_h = [np.ascontiguousarray(decoder_x[h][:, perm].astype(bf)) for h in range(H)]
    wy_h = [np.ascontiguousarray(decoder_y[h][:, perm].astype(bf)) for h in range(H)]
    enc_h = [
        np.ascontiguousarray(encoder[h * n_head + perm, :].astype(bf))
        for h in range(H)
    ]
    ro_h = [
        np.ascontiguousarray(readout[:, h * VSLICE:(h + 1) * VSLICE].astype(bf))
        for h in range(H)
    ]
    idx_b = [np.ascontiguousarray(idx[b].reshape(2, P).astype(np.int32))
             for b in range(B)]

    in_maps = []
    for c in range(8):
        b, h = c // 4, c % 4
        in_maps.append({
            "wte": wte,
            "idx2": idx_b[b],
            "wx": wx_h[h],
            "wy": wy_h[h],
            "enc": enc_h[h],
            "ro": ro_h[h],
            "cs": cs,
            "masks": masks,
            "ident": ident,
        })

    return in_maps


if __name__ == "__main__":
    nc = build_nc()
    print("built + compiled OK")


# revision 13
# speedup vs baseline: 1.0415x; 1.0415x over previous
"""Trainium2 Bass kernel for the BDH-style weight-tied transformer.

Contract: kernel(**inputs) takes FULL unsharded numpy inputs (idx, wte,
encoder, decoder_x, decoder_y, readout) and returns the FULL (B, T, V)
logits, running the model on 8 NeuronCores via run_bass_kernel_spmd.

Sharding: core c -> (b = c // 4, h = c % 4).  Group {0..3} handles batch 0,
{4..7} batch 1.  Within a group: tensor-parallel over heads with
AllGather + local-sum for (a) the head-summed attention matrix and (b) the
y @ encoder projection.  LayerNorm is scale-invariant, so summing heads
(instead of averaging) is exact.  Readout is vocab-split 4 ways per group.

The neuron axis of each head is permuted host-side so RoPE pair partners
(2k, 2k+1) live at the same partition of sibling 128-chunks ("even" chunk
2c / "odd" chunk 2c+1).  The rotation then needs no cross-partition data
movement.  The 1/sqrt(d) attention scale is folded into the cos/sin tables
(d**-0.25 on each factor of the Gram matrix).
"""

import sys

for _p in ("/opt/trn_rl_repo", "/opt/pypackages"):
    if _p not in sys.path:
        sys.path.append(_p)

import ml_dtypes
import numpy as np

import concourse.bass as bass
import concourse.mybir as mybir
import concourse.tile as tile
from concourse import bacc
from concourse.bass_utils import run_bass_kernel_spmd

F32 = mybir.dt.float32
BF16 = mybir.dt.bfloat16
I32 = mybir.dt.int32
AX = mybir.AxisListType
ALU = mybir.AluOpType
ACT = mybir.ActivationFunctionType

# Model dims (hardcoded per problem spec)
B, T, D, H, N, V = 2, 256, 256, 4, 32768, 32000
n_head = N // H            # 8192 neurons per head (one core's slice)
P = 128
NCH = n_head // P          # 64 chunks of 128 neurons
NPAIR = NCH // 2           # 32 pair-chunks
L_LAYERS = 6
LN_EPS = 1e-5
ROPE_BASE = 10000.0
VSLICE = V // 4            # 8000 vocab columns per core
VCH = 500                  # vocab chunk (PSUM bank holds 512 f32)
NVCH = VSLICE // VCH       # 16
GX = 4                     # n-chunks per streamed weight group
S4 = float(n_head) ** -0.25


def _ln_pair(nc, pools, srcs, out, out_dtype_note=""):
    """LayerNorm over the free dim (D=256) of two [128, 256] f32 tiles.

    srcs: list of 2 APs (SBUF or PSUM, f32).  out: [128, 2, 256] tile.
    """
    psmall, pcent, psq = pools["small"], pools["cent"], pools["sq"]
    for i, src in enumerate(srcs):
        ssum = psmall.tile([P, 1], F32, name=f"ln_sum{i}", tag="lnstat")
        nc.vector.tensor_reduce(ssum, src, axis=AX.X, op=ALU.add)
        negmean = psmall.tile([P, 1], F32, name=f"ln_negmean{i}", tag="lnstat")
        nc.vector.tensor_scalar_mul(negmean, ssum, -1.0 / D)
        cent = pcent.tile([P, T], F32, name=f"ln_cent{i}", tag="lncent")
        nc.vector.tensor_scalar_add(cent, src, negmean)
        sq = psq.tile([P, T], F32, name=f"ln_sq{i}", tag="lnsq")
        ss2 = psmall.tile([P, 1], F32, name=f"ln_ss2_{i}", tag="lnstat")
        nc.scalar.activation(sq, cent, ACT.Square, accum_out=ss2)
        std = psmall.tile([P, 1], F32, name=f"ln_std{i}", tag="lnstat")
        nc.scalar.activation(std, ss2, ACT.Sqrt, bias=pools["eps"][:, :1],
                             scale=1.0 / D)
        rinv = psmall.tile([P, 1], F32, name=f"ln_rinv{i}", tag="lnstat")
        nc.vector.reciprocal(rinv, std)
        nc.vector.tensor_scalar_mul(out[:, i, :], cent, rinv)


def _transpose4(nc, pools, src, dst, ident):
    """dst[:, k, 128*i:128*(i+1)] = src[:, i, 128*k:128*(k+1)].T  (bf16).

    src, dst: [128, 2, 256] bf16.  Four PE transposes + ACT copies.
    """
    pwork = pools["ps_work"]
    for i in range(2):
        for k in range(2):
            tp = pwork.tile([P, P], BF16, name=f"tp_{i}_{k}", tag="work")
            nc.tensor.transpose(tp, src[:, i, P * k:P * (k + 1)], ident)
            nc.scalar.copy(dst[:, k, P * i:P * (i + 1)], tp)


def build_nc(num_cores=8):
    nc = bacc.Bacc(
        "TRN2", target_bir_lowering=False, debug=False, num_devices=num_cores
    )

    # ---- DRAM I/O (per-core data supplied via in_maps) ----
    wte_d = nc.dram_tensor("wte", [V, D], F32, kind="ExternalInput").ap()
    idx_d = nc.dram_tensor("idx2", [2, P], I32, kind="ExternalInput").ap()
    wx_d = nc.dram_tensor("wx", [D, n_head], BF16, kind="ExternalInput").ap()
    wy_d = nc.dram_tensor("wy", [D, n_head], BF16, kind="ExternalInput").ap()
    enc_d = nc.dram_tensor("enc", [n_head, D], BF16, kind="ExternalInput").ap()
    ro_d = nc.dram_tensor("ro", [D, VSLICE], BF16, kind="ExternalInput").ap()
    cs_d = nc.dram_tensor("cs", [P, NPAIR, 2 * T], BF16, kind="ExternalInput").ap()
    masks_d = nc.dram_tensor("masks", [P, 2, T], BF16, kind="ExternalInput").ap()
    ident_d = nc.dram_tensor("ident", [P, P], BF16, kind="ExternalInput").ap()
    out_d = nc.dram_tensor("out", [T, VSLICE], F32, kind="ExternalOutput").ap()

    groups = [[0, 1, 2, 3], [4, 5, 6, 7]]

    with tile.TileContext(nc) as tc:
        with (
            tc.tile_pool(name="pers", bufs=1) as pers,
            tc.tile_pool(name="pv", bufs=2) as pv,
            tc.tile_pool(name="pbig", bufs=2) as pbig,
            tc.tile_pool(name="pwx", bufs=2) as pwx,
            tc.tile_pool(name="pwy", bufs=2) as pwy,
            tc.tile_pool(name="pro", bufs=3) as pro,
            tc.tile_pool(name="pxr", bufs=6) as pxr,
            tc.tile_pool(name="py", bufs=6) as py,
            tc.tile_pool(name="psmall", bufs=12) as psmall,
            tc.tile_pool(name="pcent", bufs=4) as pcent,
            tc.tile_pool(name="psq", bufs=2) as psq,
            tc.tile_pool(name="pexp", bufs=8) as pexp,
            tc.tile_pool(name="ps_work", bufs=4, space="PSUM") as ps_work,
            tc.tile_pool(name="ps_accum", bufs=2, space="PSUM") as ps_accum,
            tc.tile_pool(name="dram", bufs=2, space="DRAM") as dram,
        ):
            pools = {
                "small": psmall,
                "cent": pcent,
                "sq": psq,
                "ps_work": ps_work,
            }

            # ---- persistent SBUF tensors ----
            eps_sb = pers.tile([P, 1], F32, name="eps_sb", tag="eps")
            nc.vector.memset(eps_sb, LN_EPS)
            pools["eps"] = eps_sb
            cs_sb = pers.tile([P, NPAIR, 2 * T], BF16, name="cs_sb", tag="cs")
            masks_sb = pers.tile([P, 2, T], BF16, name="masks_sb", tag="masks")
            ident_sb = pers.tile([P, P], BF16, name="ident_sb", tag="ident")
            enc_sb = pers.tile([P, NCH, T], BF16, name="enc_sb", tag="enc")
            x_sb = pers.tile([P, NCH, T], BF16, name="x_sb", tag="x")

            for g in range(4):
                nc.sync.dma_start(
                    cs_sb[:, 8 * g:8 * (g + 1), :], cs_d[:, 8 * g:8 * (g + 1), :]
                )
            nc.sync.dma_start(masks_sb[:], masks_d[:])
            nc.sync.dma_start(ident_sb[:], ident_d[:])
            enc_r = enc_d.rearrange("(c p) d -> p c d", p=P)
            for g in range(8):
                nc.sync.dma_start(
                    enc_sb[:, 8 * g:8 * (g + 1), :], enc_r[:, 8 * g:8 * (g + 1), :]
                )

            # ---- embedding gather + first LN ----
            vraw = pbig.tile([P, 2, T], F32, name="vraw", tag="vraw")
            for i in range(2):
                idx_sb = psmall.tile([P, 1], I32, name=f"idx_sb{i}", tag="idx")
                nc.sync.dma_start(idx_sb, idx_d[i, :].rearrange("(p o) -> p o", o=1))
                nc.gpsimd.indirect_dma_start(
                    out=vraw[:, i, :],
                    out_offset=None,
                    in_=wte_d[:],
                    in_offset=bass.IndirectOffsetOnAxis(ap=idx_sb[:, :1], axis=0),
                )
            v = pv.tile([P, 2, T], F32, name="v_l0", tag="v")
            _ln_pair(nc, pools, [vraw[:, 0, :], vraw[:, 1, :]], v)

            for layer in range(L_LAYERS):
                # ---- v_bf (natural, bf16) and vT (transposed, bf16) ----
                v_bf = pbig.tile([P, 2, T], BF16, name=f"vbf_{layer}", tag="vbf")
                for i in range(2):
                    nc.vector.tensor_copy(v_bf[:, i, :], v[:, i, :])
                vT = pbig.tile([P, 2, T], BF16, name=f"vT_{layer}", tag="vT")
                _transpose4(nc, pools, v_bf, vT, ident_sb)

                # ---- x phase: x = relu(v @ Wx), rope, scores (Gram) ----
                sc0 = ps_accum.tile([P, P], F32, name=f"sc0_{layer}", tag="acc0")
                sc1 = ps_accum.tile([P, T], F32, name=f"sc1_{layer}", tag="acc1")
                scores = [sc0, sc1]
                for grp in range(NPAIR // 2):  # 16 groups of 2 pair-chunks
                    ch0 = 4 * grp  # first of 4 n-chunks in this group
                    if ch0 % GX == 0:
                        wxg = pwx.tile([P, 2, GX * P], BF16,
                                       name=f"wxg_{layer}_{ch0}", tag="wx")
                        for dk in range(2):
                            nc.sync.dma_start(
                                wxg[:, dk, :],
                                wx_d[P * dk:P * (dk + 1),
                                     P * ch0:P * (ch0 + GX)],
                            )
                    for pc in (2 * grp, 2 * grp + 1):
                        x_pre = ps_work.tile([P, 2 * T], F32,
                                             name=f"xpre_{layer}_{pc}", tag="work")
                        for m in range(2):  # even / odd member chunk
                            ch = 2 * pc + m
                            co = P * (ch % GX)
                            for dk in range(2):
                                nc.tensor.matmul(
                                    x_pre[:, T * m:T * (m + 1)],
                                    lhsT=wxg[:, dk, co:co + P],
                                    rhs=vT[:, dk, :],
                                    start=(dk == 0),
                                    stop=(dk == 1),
                                )
                        nc.scalar.activation(
                            x_sb[:, 2 * pc:2 * pc + 2, :], x_pre, ACT.Relu)
                    # rope over the 2 pair-chunks (even chunks ch0, ch0+2;
                    # odd chunks ch0+1, ch0+3), batched FD=512
                    xe = x_sb[:, ch0:ch0 + 4:2, :]
                    xo = x_sb[:, ch0 + 1:ch0 + 4:2, :]
                    cvw = cs_sb[:, 2 * grp:2 * grp + 2, 0:T]
                    svw = cs_sb[:, 2 * grp:2 * grp + 2, T:2 * T]
                    m_ec = pxr.tile([P, 2, T], BF16, name=f"mec_{layer}_{grp}", tag="xr")
                    m_os = pxr.tile([P, 2, T], BF16, name=f"mos_{layer}_{grp}", tag="xr")
                    m_oc = pxr.tile([P, 2, T], BF16, name=f"moc_{layer}_{grp}", tag="xr")
                    m_es = pxr.tile([P, 2, T], BF16, name=f"mes_{layer}_{grp}", tag="xr")
                    xr_e = pxr.tile([P, 2, T], BF16, name=f"xre_{layer}_{grp}", tag="xr")
                    xr_o = pxr.tile([P, 2, T], BF16, name=f"xro_{layer}_{grp}", tag="xr")
                    nc.vector.tensor_mul(m_ec, xe, cvw)
                    nc.vector.tensor_mul(m_os, xo, svw)
                    nc.vector.tensor_sub(xr_e, m_ec, m_os)
                    nc.vector.tensor_mul(m_oc, xo, cvw)
                    nc.vector.tensor_mul(m_es, xe, svw)
                    nc.vector.tensor_add(xr_o, m_oc, m_es)
                    for q in range(2):  # pair-chunk within group
                        for xr in (xr_e, xr_o):
                            chv = ch0 + 2 * q + (0 if xr is xr_e else 1)
                            nc.tensor.matmul(
                                scores[0],
                                lhsT=xr[:, q, 0:P],
                                rhs=xr[:, q, 0:P],
                                start=(chv == 0),
                                stop=(chv == NCH - 1),
                            )
                            nc.tensor.matmul(
                                scores[1],
                                lhsT=xr[:, q, P:2 * P],
                                rhs=xr[:, q, :],
                                start=(chv == 0),
                                stop=(chv == NCH - 1),
                            )

                # ---- softmax (causal, per-head normalized) ----
                # attn packed [128, 384]: cols 0:128 = t-tile0 (s<128),
                # cols 128:384 = t-tile1 (s<256)
                attn = pexp.tile([P, 3 * P], BF16, name=f"attn_{layer}", tag="attn", bufs=2)
                for i, (w, lo) in enumerate(((P, 0), (T, P))):
                    mx = psmall.tile([P, 1], F32, name=f"mx_{i}", tag="lnstat")
                    nc.vector.tensor_reduce(mx, scores[i], axis=AX.X, op=ALU.max)
                    negmx = psmall.tile([P, 1], F32, name=f"negmx_{i}", tag="lnstat")
                    nc.vector.tensor_scalar_mul(negmx, mx, -1.0)
                    ex = pexp.tile([P, w], BF16, name=f"ex_{layer}_{i}", tag="ex", bufs=2)
                    nc.scalar.activation(ex, scores[i], ACT.Exp, bias=negmx)
                    nc.vector.tensor_mul(ex, ex, masks_sb[:, i, 0:w])
                    rs = psmall.tile([P, 1], F32, name=f"rs_{i}", tag="lnstat")
                    nc.vector.tensor_reduce(rs, ex, axis=AX.X, op=ALU.add)
                    rcp = psmall.tile([P, 1], F32, name=f"rcp_{i}", tag="lnstat")
                    nc.vector.reciprocal(rcp, rs)
                    nc.vector.tensor_scalar_mul(attn[:, lo:lo + w], ex, rcp)

                # ---- AllGather attn over the 4-core group; sum heads ----
                attn_bnc = dram.tile([P, 3 * P], BF16,
                                     name=f"attn_bnc_{layer}", tag="attn_in")
                nc.gpsimd.dma_start(attn_bnc[:], attn[:])
                attn_gth = dram.tile([4 * P, 3 * P], BF16, name=f"attn_gth_{layer}",
                                     tag="attn_out")
                nc.gpsimd.collective_compute(
                    "AllGather", ALU.bypass, replica_groups=groups,
                    ins=[attn_bnc.opt()], outs=[attn_gth.opt()],
                )
                asum = pexp.tile([P, 3 * P], BF16, name=f"asum_{layer}", tag="asum", bufs=2)
                g_t = []
                for g in range(4):
                    gt = pexp.tile([P, 3 * P], BF16,
                                   name=f"ag_{layer}_{g}", tag="ag", bufs=8)
                    nc.sync.dma_start(gt, attn_gth[P * g:P * (g + 1), :])
                    g_t.append(gt)
                s01 = pexp.tile([P, 3 * P], BF16, name="s01", tag="ag", bufs=8)
                s23 = pexp.tile([P, 3 * P], BF16, name="s23", tag="ag", bufs=8)
                nc.vector.tensor_add(s01, g_t[0], g_t[1])
                nc.vector.tensor_add(s23, g_t[2], g_t[3])
                nc.vector.tensor_add(asum, s01, s23)

                # ---- transpose summed attn; a = attnT.T @ v; LN(a) ----
                # attnT blocks: b00 = attn[t0, s0].T; b10/b11 = attn[t1, :].T
                attnT = pexp.tile([P, 3 * P], BF16, name=f"attnT_{layer}", tag="attnT", bufs=2)
                for bi, (alo, tlo) in enumerate(((0, 0), (P, P), (2 * P, 2 * P))):
                    tp = ps_work.tile([P, P], BF16, name=f"tpa_{bi}", tag="work")
                    nc.tensor.transpose(tp, asum[:, alo:alo + P], ident_sb)
                    nc.scalar.copy(attnT[:, tlo:tlo + P], tp)
                a_ps = []
                ap_0 = ps_work.tile([P, T], F32, name=f"aps_{layer}_0", tag="work")
                nc.tensor.matmul(ap_0, lhsT=attnT[:, 0:P], rhs=v_bf[:, 0, :],
                                 start=True, stop=True)
                a_ps.append(ap_0)
                ap_1 = ps_work.tile([P, T], F32, name=f"aps_{layer}_1", tag="work")
                for j in range(2):
                    nc.tensor.matmul(
                        ap_1,
                        lhsT=attnT[:, P * (1 + j):P * (2 + j)],
                        rhs=v_bf[:, j, :],
                        start=(j == 0),
                        stop=(j == 1),
                    )
                a_ps.append(ap_1)
                lnA = pbig.tile([P, 2, T], BF16, name=f"lnA_{layer}", tag="lnA")
                _ln_pair(nc, pools, a_ps, lnA)
                lnAT = pbig.tile([P, 2, T], BF16, name=f"lnAT_{layer}", tag="lnAT")
                _transpose4(nc, pools, lnA, lnAT, ident_sb)

                # ---- y phase: y = relu(lnA @ Wy) * x;  yenc = y @ enc ----
                ye0 = ps_accum.tile([P, T], F32, name=f"ye0_{layer}", tag="acc0")
                ye1 = ps_accum.tile([P, T], F32, name=f"ye1_{layer}", tag="acc1")
                yenc = [ye0, ye1]
                for pc in range(NCH // 2):  # two n-chunks at a time
                    ch0y = 2 * pc
                    if ch0y % GX == 0:
                        wyg = pwy.tile([P, 2, GX * P], BF16,
                                       name=f"wyg_{layer}_{ch0y}", tag="wy")
                        for dk in range(2):
                            nc.sync.dma_start(
                                wyg[:, dk, :],
                                wy_d[P * dk:P * (dk + 1), P * ch0y:P * (ch0y + GX)],
                            )
                    y_pre = ps_work.tile([P, 2 * T], F32, name=f"ypre_{layer}_{pc}",
                                         tag="work")
                    for m in range(2):
                        co = P * ((ch0y + m) % GX)
                        for dk in range(2):
                            nc.tensor.matmul(
                                y_pre[:, T * m:T * (m + 1)],
                                lhsT=wyg[:, dk, co:co + P],
                                rhs=lnAT[:, dk, :],
                                start=(dk == 0),
                                stop=(dk == 1),
                            )
                    yr = py.tile([P, 2 * T], BF16, name=f"yr_{layer}_{pc}", tag="y")
                    nc.scalar.activation(yr, y_pre, ACT.Relu)
                    yt = py.tile([P, 2 * T], BF16, name=f"yt_{layer}_{pc}", tag="y")
                    nc.vector.tensor_mul(yt, yr, x_sb[:, ch0y:ch0y + 2, :])
                    for m in range(2):
                        for i in range(2):
                            nc.tensor.matmul(
                                yenc[i],
                                lhsT=yt[:, T * m + P * i:T * m + P * (i + 1)],
                                rhs=enc_sb[:, ch0y + m, :],
                                start=(ch0y + m == 0),
                                stop=(ch0y + m == NCH - 1),
                            )

                # ---- AllGather yenc partials (f32) + sum + LNs + residual ----
                ye_bnc = dram.tile([P, 2 * T], BF16, name=f"ye_bnc_{layer}",
                                   tag="ye_in")
                ye_sb = pexp.tile([P, 2 * T], BF16, name=f"ye_sb_{layer}",
                                  tag="yg", bufs=8)
                for i in range(2):
                    nc.scalar.copy(ye_sb[:, T * i:T * (i + 1)], yenc[i])
                nc.sync.dma_start(ye_bnc[:], ye_sb)
                ye_gth = dram.tile([4 * P, 2 * T], BF16, name=f"ye_gth_{layer}",
                                   tag="ye_out")
                nc.gpsimd.collective_compute(
                    "AllGather", ALU.bypass, replica_groups=groups,
                    ins=[ye_bnc.opt()], outs=[ye_gth.opt()],
                )
                ysum = pbig.tile([P, 2, T], F32, name=f"ysum_{layer}", tag="ysum")
                g_t = []
                for g in range(4):
                    gt = pexp.tile([P, 2 * T], BF16, name=f"yg_{layer}_{g}",
                                   tag="yg", bufs=8)
                    nc.sync.dma_start(gt, ye_gth[P * g:P * (g + 1), :])
                    g_t.append(gt)
                ys01 = pexp.tile([P, 2 * T], BF16, name="ys01", tag="yg", bufs=8)
                ys23 = pexp.tile([P, 2 * T], BF16, name="ys23", tag="yg", bufs=8)
                nc.vector.tensor_add(ys01, g_t[0], g_t[1])
                nc.vector.tensor_add(ys23, g_t[2], g_t[3])
                nc.vector.tensor_add(
                    ysum.rearrange("p a t -> p (a t)"), ys01, ys23)
                lnY = pbig.tile([P, 2, T], F32, name=f"lnY_{layer}", tag="lnY")
                _ln_pair(nc, pools, [ysum[:, 0, :], ysum[:, 1, :]], lnY)
                vres = pbig.tile([P, 2, T], F32, name=f"vres_{layer}", tag="vres")
                for i in range(2):
                    nc.vector.tensor_add(vres[:, i, :], v[:, i, :], lnY[:, i, :])
                v = pv.tile([P, 2, T], F32, name=f"v_l{layer + 1}", tag="v")
                _ln_pair(nc, pools, [vres[:, 0, :], vres[:, 1, :]], v)

            # ---- readout: out = v @ ro  (vocab slice) ----
            v_bf = pbig.tile([P, 2, T], BF16, name="vbf_ro", tag="vbf")
            for i in range(2):
                nc.vector.tensor_copy(v_bf[:, i, :], v[:, i, :])
            vT = pbig.tile([P, 2, T], BF16, name="vT_ro", tag="vT")
            _transpose4(nc, pools, v_bf, vT, ident_sb)
            for c in range(NVCH):
                rog = pro.tile([P, 2, VCH], BF16, name=f"rog_{c}", tag="ro")
                for dk in range(2):
                    nc.sync.dma_start(
                        rog[:, dk, :],
                        ro_d[P * dk:P * (dk + 1), VCH * c:VCH * (c + 1)],
                    )
                for i in range(2):
                    lg = ps_work.tile([P, VCH], F32, name=f"lg_{c}_{i}", tag="work")
                    for dk in range(2):
                        nc.tensor.matmul(
                            lg,
                            lhsT=vT[:, dk, P * i:P * (i + 1)],
                            rhs=rog[:, dk, :],
                            start=(dk == 0),
                            stop=(dk == 1),
                        )
                    lg_sb = py.tile([P, VCH], F32, name=f"lg_sb_{c}_{i}",
                                    tag="lgsb", bufs=4)
                    nc.vector.tensor_copy(lg_sb, lg)
                    nc.sync.dma_start(
                        out_d[P * i:P * (i + 1), VCH * c:VCH * (c + 1)], lg_sb
                    )

    nc.compile()
    return nc


# ------------------------- host-side preparation -------------------------

def _pair_perm():
    """perm[new] = old index within a head, de-interleaving rope pairs."""
    perm = np.zeros(n_head, dtype=np.int64)
    for c in range(NPAIR):
        k = np.arange(P) + c * P          # pair indices in this pair-chunk
        perm[(2 * c) * P + np.arange(P)] = 2 * k
        perm[(2 * c + 1) * P + np.arange(P)] = 2 * k + 1
    return perm


def _rope_tables():
    """cs[p, c, 0:T] = cos, cs[p, c, T:2T] = sin, scaled by d**-0.25."""
    inv_freq = 1.0 / (
        ROPE_BASE ** (np.arange(0, n_head, 2, dtype=np.float32) / n_head)
    )  # (4096,) f32, matching reference arithmetic
    t = np.arange(T, dtype=np.float32)
    freqs = t[:, None] * inv_freq[None, :]         # (T, 4096) f32
    cos = np.cos(freqs) * S4                       # (T, 4096)
    sin = np.sin(freqs) * S4
    cs = np.zeros((P, NPAIR, 2 * T), dtype=np.float32)
    for c in range(NPAIR):
        k = c * P + np.arange(P)                   # (128,) pair indices
        cs[:, c, 0:T] = cos[:, k].T
        cs[:, c, T:2 * T] = sin[:, k].T
    return cs.astype(ml_dtypes.bfloat16)


def _masks():
    # [P, 2, T]: tile0 mask in [:, 0, 0:128] (s<=t); tile1 in [:, 1, 0:256]
    m = np.zeros((P, 2, T), dtype=np.float32)
    t = np.arange(P)[:, None]
    m[:, 0, 0:P] = (np.arange(P)[None, :] <= t).astype(np.float32)
    m[:, 1, :] = (np.arange(T)[None, :] <= t + P).astype(np.float32)
    return m.astype(ml_dtypes.bfloat16)


_CACHE = {}


def kernel(idx, wte, encoder, decoder_x, decoder_y, readout):
    if "nc" not in _CACHE:
        _CACHE["nc"] = build_nc()
    nc = _CACHE["nc"]
    in_maps = prepare_in_maps(idx, wte, encoder, decoder_x, decoder_y, readout)
    res = run_bass_kernel_spmd(nc, in_maps, core_ids=list(range(8)))
    return assemble_output([res.results[c]["out"] for c in range(8)])


def assemble_output(outs):
    out = np.empty((B, T, V), dtype=np.float32)
    for c in range(8):
        b, h = c // 4, c % 4
        out[b, :, h * VSLICE:(h + 1) * VSLICE] = outs[c]
    return out


def prepare_in_maps(idx, wte, encoder, decoder_x, decoder_y, readout):
    idx = np.asarray(idx)
    wte = np.ascontiguousarray(np.asarray(wte, dtype=np.float32))
    encoder = np.asarray(encoder, dtype=np.float32)
    decoder_x = np.asarray(decoder_x, dtype=np.float32)
    decoder_y = np.asarray(decoder_y, dtype=np.float32)
    readout = np.asarray(readout, dtype=np.float32)

    perm = _pair_perm()
    cs = _rope_tables()
    masks = _masks()
    ident = np.eye(P, dtype=np.float32).astype(ml_dtypes.bfloat16)
    bf = ml_dtypes.bfloat16

    wx_h = [np.ascontiguousarray(decoder_x[h][:, perm].astype(bf)) for h in range(H)]
    wy_h = [np.ascontiguousarray(decoder_y[h][:, perm].astype(bf)) for h in range(H)]
    enc_h = [
        np.ascontiguousarray(encoder[h * n_head + perm, :].astype(bf))
        for h in range(H)
    ]
    ro_h = [
        np.ascontiguousarray(readout[:, h * VSLICE:(h + 1) * VSLICE].astype(bf))
        for h in range(H)
    ]
    idx_b = [np.ascontiguousarray(idx[b].reshape(2, P).astype(np.int32))
             for b in range(B)]

    in_maps = []
    for c in range(8):
        b, h = c // 4, c % 4
        in_maps.append({
            "wte": wte,
            "idx2": idx_b[b],
            "wx": wx_h[h],
            "wy": wy_h[h],
            "enc": enc_h[h],
            "ro": ro_h[h],
            "cs": cs,
            "masks": masks,
            "ident": ident,
        })

    return in_maps


if __name__ == "__main__":
    nc = build_nc()
    print("built + compiled OK")


# revision 15
# speedup vs baseline: 1.1878x; 1.1405x over previous
"""Trainium2 Bass kernel for the BDH-style weight-tied transformer.

Contract: kernel(**inputs) takes FULL unsharded numpy inputs (idx, wte,
encoder, decoder_x, decoder_y, readout) and returns the FULL (B, T, V)
logits, running the model on 8 NeuronCores via run_bass_kernel_spmd.

Sharding: core c -> (b = c // 4, h = c % 4).  Group {0..3} handles batch 0,
{4..7} batch 1.  Within a group: tensor-parallel over heads with
AllGather + local-sum for (a) the head-summed attention matrix and (b) the
y @ encoder projection.  LayerNorm is scale-invariant, so summing heads
(instead of averaging) is exact.  Readout is vocab-split 4 ways per group.

The neuron axis of each head is permuted host-side so RoPE pair partners
(2k, 2k+1) live at the same partition of sibling 128-chunks ("even" chunk
2c / "odd" chunk 2c+1).  The rotation then needs no cross-partition data
movement.  The 1/sqrt(d) attention scale is folded into the cos/sin tables
(d**-0.25 on each factor of the Gram matrix).
"""

import sys

for _p in ("/opt/trn_rl_repo", "/opt/pypackages"):
    if _p not in sys.path:
        sys.path.append(_p)

import ml_dtypes
import numpy as np

import concourse.bass as bass
import concourse.mybir as mybir
import concourse.tile as tile
from concourse import bacc
from concourse.bass_utils import run_bass_kernel_spmd

F32 = mybir.dt.float32
BF16 = mybir.dt.bfloat16
I32 = mybir.dt.int32
AX = mybir.AxisListType
ALU = mybir.AluOpType
ACT = mybir.ActivationFunctionType

# Model dims (hardcoded per problem spec)
B, T, D, H, N, V = 2, 256, 256, 4, 32768, 32000
n_head = N // H            # 8192 neurons per head (one core's slice)
P = 128
NCH = n_head // P          # 64 chunks of 128 neurons
NPAIR = NCH // 2           # 32 pair-chunks
L_LAYERS = 6
LN_EPS = 1e-5
ROPE_BASE = 10000.0
VSLICE = V // 4            # 8000 vocab columns per core
VCH = 500                  # vocab chunk (PSUM bank holds 512 f32)
NVCH = VSLICE // VCH       # 16
GX = 4                     # n-chunks per streamed weight group
S4 = float(n_head) ** -0.25


def _ln_pair(nc, pools, srcs, out, out_dtype_note=""):
    """LayerNorm over the free dim (D=256) of two [128, 256] f32 tiles.

    srcs: list of 2 APs (SBUF or PSUM, f32).  out: [128, 2, 256] tile.
    """
    psmall = pools["small"]
    for i, src in enumerate(srcs):
        stats = psmall.tile([P, 6], F32, name=f"ln_st{i}", tag="lnstat")
        nc.vector.bn_stats(stats, src)
        aggr = psmall.tile([P, 2], F32, name=f"ln_ag{i}", tag="lnstat")
        nc.vector.bn_aggr(aggr, stats)
        std = psmall.tile([P, 1], F32, name=f"ln_std{i}", tag="lnstat")
        nc.scalar.activation(std, aggr[:, 1:2], ACT.Sqrt,
                             bias=pools["eps"][:, :1])
        rinv = psmall.tile([P, 1], F32, name=f"ln_rinv{i}", tag="lnstat")
        nc.vector.reciprocal(rinv, std)
        nc.vector.tensor_scalar(out[:, i, :], src, aggr[:, 0:1], rinv,
                                op0=ALU.subtract, op1=ALU.mult)


def _transpose4(nc, pools, src, dst, ident):
    """dst[:, k, 128*i:128*(i+1)] = src[:, i, 128*k:128*(k+1)].T  (bf16).

    src, dst: [128, 2, 256] bf16.  Four PE transposes + ACT copies.
    """
    pwork = pools["ps_work"]
    for i in range(2):
        for k in range(2):
            tp = pwork.tile([P, P], BF16, name=f"tp_{i}_{k}", tag="work")
            nc.tensor.transpose(tp, src[:, i, P * k:P * (k + 1)], ident)
            nc.scalar.copy(dst[:, k, P * i:P * (i + 1)], tp)


def build_nc(num_cores=8):
    nc = bacc.Bacc(
        "TRN2", target_bir_lowering=False, debug=False, num_devices=num_cores
    )

    # ---- DRAM I/O (per-core data supplied via in_maps) ----
    wte_d = nc.dram_tensor("wte", [V, D], F32, kind="ExternalInput").ap()
    idx_d = nc.dram_tensor("idx2", [2, P], I32, kind="ExternalInput").ap()
    wx_d = nc.dram_tensor("wx", [D, n_head], BF16, kind="ExternalInput").ap()
    wy_d = nc.dram_tensor("wy", [D, n_head], BF16, kind="ExternalInput").ap()
    enc_d = nc.dram_tensor("enc", [n_head, D], BF16, kind="ExternalInput").ap()
    ro_d = nc.dram_tensor("ro", [D, VSLICE], BF16, kind="ExternalInput").ap()
    cs_d = nc.dram_tensor("cs", [P, NPAIR, 2 * T], BF16, kind="ExternalInput").ap()
    masks_d = nc.dram_tensor("masks", [P, 2, T], BF16, kind="ExternalInput").ap()
    ident_d = nc.dram_tensor("ident", [P, P], BF16, kind="ExternalInput").ap()
    out_d = nc.dram_tensor("out", [T, VSLICE], F32, kind="ExternalOutput").ap()

    groups = [[0, 1, 2, 3], [4, 5, 6, 7]]

    with tile.TileContext(nc) as tc:
        with (
            tc.tile_pool(name="pers", bufs=1) as pers,
            tc.tile_pool(name="pv", bufs=2) as pv,
            tc.tile_pool(name="pbig", bufs=2) as pbig,
            tc.tile_pool(name="pwx", bufs=3) as pwx,
            tc.tile_pool(name="pwy", bufs=3) as pwy,
            tc.tile_pool(name="pro", bufs=3) as pro,
            tc.tile_pool(name="pxr", bufs=8) as pxr,
            tc.tile_pool(name="py", bufs=6) as py,
            tc.tile_pool(name="psmall", bufs=12) as psmall,
            tc.tile_pool(name="pcent", bufs=4) as pcent,
            tc.tile_pool(name="psq", bufs=2) as psq,
            tc.tile_pool(name="pexp", bufs=8) as pexp,
            tc.tile_pool(name="ps_work", bufs=5, space="PSUM") as ps_work,
            tc.tile_pool(name="ps_accum", bufs=1, space="PSUM") as ps_accum,
            tc.tile_pool(name="dram", bufs=2, space="DRAM") as dram,
        ):
            pools = {
                "small": psmall,
                "cent": pcent,
                "sq": psq,
                "ps_work": ps_work,
            }

            # ---- persistent SBUF tensors ----
            eps_sb = pers.tile([P, 1], F32, name="eps_sb", tag="eps")
            nc.vector.memset(eps_sb, LN_EPS)
            pools["eps"] = eps_sb
            cs_sb = pers.tile([P, NPAIR, 2 * T], BF16, name="cs_sb", tag="cs")
            masks_sb = pers.tile([P, 2, T], BF16, name="masks_sb", tag="masks")
            ident_sb = pers.tile([P, P], BF16, name="ident_sb", tag="ident")
            enc_sb = pers.tile([P, NCH, T], BF16, name="enc_sb", tag="enc")
            x_sb = pers.tile([P, NCH, T], BF16, name="x_sb", tag="x")

            for g in range(4):
                nc.sync.dma_start(
                    cs_sb[:, 8 * g:8 * (g + 1), :], cs_d[:, 8 * g:8 * (g + 1), :]
                )
            nc.sync.dma_start(masks_sb[:], masks_d[:])
            nc.sync.dma_start(ident_sb[:], ident_d[:])
            enc_r = enc_d.rearrange("(c p) d -> p c d", p=P)
            for g in range(8):
                nc.sync.dma_start(
                    enc_sb[:, 8 * g:8 * (g + 1), :], enc_r[:, 8 * g:8 * (g + 1), :]
                )

            # ---- embedding gather + first LN ----
            vraw = pbig.tile([P, 2, T], F32, name="vraw", tag="vraw")
            for i in range(2):
                idx_sb = psmall.tile([P, 1], I32, name=f"idx_sb{i}", tag="idx")
                nc.sync.dma_start(idx_sb, idx_d[i, :].rearrange("(p o) -> p o", o=1))
                nc.gpsimd.indirect_dma_start(
                    out=vraw[:, i, :],
                    out_offset=None,
                    in_=wte_d[:],
                    in_offset=bass.IndirectOffsetOnAxis(ap=idx_sb[:, :1], axis=0),
                )
            v = pv.tile([P, 2, T], F32, name="v_l0", tag="v")
            _ln_pair(nc, pools, [vraw[:, 0, :], vraw[:, 1, :]], v)

            for layer in range(L_LAYERS):
                # ---- v_bf (natural, bf16) and vT (transposed, bf16) ----
                v_bf = pbig.tile([P, 2, T], BF16, name=f"vbf_{layer}", tag="vbf")
                for i in range(2):
                    nc.vector.tensor_copy(v_bf[:, i, :], v[:, i, :])
                vT = pbig.tile([P, 2, T], BF16, name=f"vT_{layer}", tag="vT")
                _transpose4(nc, pools, v_bf, vT, ident_sb)

                # ---- x phase: x = relu(v @ Wx), rope, scores (Gram) ----
                sc0 = ps_accum.tile([P, P], F32, name=f"sc0_{layer}", tag="acc0")
                sc1 = ps_accum.tile([P, T], F32, name=f"sc1_{layer}", tag="acc1")
                scores = [sc0, sc1]
                for grp in range(NPAIR // 2):  # 16 groups of 2 pair-chunks
                    ch0 = 4 * grp  # first of 4 n-chunks in this group
                    if ch0 % GX == 0:
                        wxg = pwx.tile([P, 2, GX * P], BF16,
                                       name=f"wxg_{layer}_{ch0}", tag="wx")
                        for dk in range(2):
                            nc.sync.dma_start(
                                wxg[:, dk, :],
                                wx_d[P * dk:P * (dk + 1),
                                     P * ch0:P * (ch0 + GX)],
                            )
                    for pc in (2 * grp, 2 * grp + 1):
                        x_pre = ps_work.tile([P, 2 * T], F32,
                                             name=f"xpre_{layer}_{pc}", tag="work")
                        for m in range(2):  # even / odd member chunk
                            ch = 2 * pc + m
                            co = P * (ch % GX)
                            for dk in range(2):
                                nc.tensor.matmul(
                                    x_pre[:, T * m:T * (m + 1)],
                                    lhsT=wxg[:, dk, co:co + P],
                                    rhs=vT[:, dk, :],
                                    start=(dk == 0),
                                    stop=(dk == 1),
                                )
                        nc.scalar.activation(
                            x_sb[:, 2 * pc:2 * pc + 2, :], x_pre, ACT.Relu)
                    # rope over the 2 pair-chunks (even chunks ch0, ch0+2;
                    # odd chunks ch0+1, ch0+3), batched FD=512
                    xe = x_sb[:, ch0:ch0 + 4:2, :]
                    xo = x_sb[:, ch0 + 1:ch0 + 4:2, :]
                    cvw = cs_sb[:, 2 * grp:2 * grp + 2, 0:T]
                    svw = cs_sb[:, 2 * grp:2 * grp + 2, T:2 * T]
                    m_ec = pxr.tile([P, 2, T], BF16, name=f"mec_{layer}_{grp}", tag="xr")
                    m_os = pxr.tile([P, 2, T], BF16, name=f"mos_{layer}_{grp}", tag="xr")
                    m_oc = pxr.tile([P, 2, T], BF16, name=f"moc_{layer}_{grp}", tag="xr")
                    m_es = pxr.tile([P, 2, T], BF16, name=f"mes_{layer}_{grp}", tag="xr")
                    xr_e = pxr.tile([P, 2, T], BF16, name=f"xre_{layer}_{grp}", tag="xr")
                    xr_o = pxr.tile([P, 2, T], BF16, name=f"xro_{layer}_{grp}", tag="xr")
                    nc.vector.tensor_mul(m_ec, xe, cvw)
                    nc.vector.tensor_mul(m_os, xo, svw)
                    nc.vector.tensor_sub(xr_e, m_ec, m_os)
                    nc.vector.tensor_mul(m_oc, xo, cvw)
                    nc.vector.tensor_mul(m_es, xe, svw)
                    nc.vector.tensor_add(xr_o, m_oc, m_es)
                    for q in range(2):  # pair-chunk within group
                        for xr in (xr_e, xr_o):
                            chv = ch0 + 2 * q + (0 if xr is xr_e else 1)
                            nc.tensor.matmul(
                                scores[0],
                                lhsT=xr[:, q, 0:P],
                                rhs=xr[:, q, 0:P],
                                start=(chv == 0),
                                stop=(chv == NCH - 1),
                            )
                            nc.tensor.matmul(
                                scores[1],
                                lhsT=xr[:, q, P:2 * P],
                                rhs=xr[:, q, :],
                                start=(chv == 0),
                                stop=(chv == NCH - 1),
                            )

                # ---- softmax (causal, per-head normalized) ----
                # attn packed [128, 384]: cols 0:128 = t-tile0 (s<128),
                # cols 128:384 = t-tile1 (s<256)
                attn = pexp.tile([P, 3 * P], BF16, name=f"attn_{layer}", tag="attn", bufs=2)
                for i, (w, lo) in enumerate(((P, 0), (T, P))):
                    mx = psmall.tile([P, 1], F32, name=f"mx_{i}", tag="lnstat")
                    nc.vector.tensor_reduce(mx, scores[i], axis=AX.X, op=ALU.max)
                    negmx = psmall.tile([P, 1], F32, name=f"negmx_{i}", tag="lnstat")
                    nc.vector.tensor_scalar_mul(negmx, mx, -1.0)
                    ex = pexp.tile([P, w], BF16, name=f"ex_{layer}_{i}", tag="ex", bufs=2)
                    nc.scalar.activation(ex, scores[i], ACT.Exp, bias=negmx)
                    nc.vector.tensor_mul(ex, ex, masks_sb[:, i, 0:w])
                    rs = psmall.tile([P, 1], F32, name=f"rs_{i}", tag="lnstat")
                    nc.vector.tensor_reduce(rs, ex, axis=AX.X, op=ALU.add)
                    rcp = psmall.tile([P, 1], F32, name=f"rcp_{i}", tag="lnstat")
                    nc.vector.reciprocal(rcp, rs)
                    nc.vector.tensor_scalar_mul(attn[:, lo:lo + w], ex, rcp)

                # ---- AllGather attn over the 4-core group; sum heads ----
                attn_bnc = dram.tile([P, 3 * P], BF16,
                                     name=f"attn_bnc_{layer}", tag="attn_in")
                nc.gpsimd.dma_start(attn_bnc[:], attn[:])
                attn_gth = dram.tile([P, 3 * P], BF16, name=f"attn_gth_{layer}",
                                     tag="attn_out")
                nc.gpsimd.collective_compute(
                    "AllReduce", ALU.add, replica_groups=groups,
                    ins=[attn_bnc.opt()], outs=[attn_gth.opt()],
                )
                asum = pexp.tile([P, 3 * P], BF16, name=f"asum_{layer}", tag="asum", bufs=2)
                nc.sync.dma_start(asum, attn_gth[:])

                # ---- transpose summed attn; a = attnT.T @ v; LN(a) ----
                # attnT blocks: b00 = attn[t0, s0].T; b10/b11 = attn[t1, :].T
                attnT = pexp.tile([P, 3 * P], BF16, name=f"attnT_{layer}", tag="attnT", bufs=2)
                for bi, (alo, tlo) in enumerate(((0, 0), (P, P), (2 * P, 2 * P))):
                    tp = ps_work.tile([P, P], BF16, name=f"tpa_{bi}", tag="work")
                    nc.tensor.transpose(tp, asum[:, alo:alo + P], ident_sb)
                    nc.scalar.copy(attnT[:, tlo:tlo + P], tp)
                a_ps = []
                ap_0 = ps_work.tile([P, T], F32, name=f"aps_{layer}_0", tag="work")
                nc.tensor.matmul(ap_0, lhsT=attnT[:, 0:P], rhs=v_bf[:, 0, :],
                                 start=True, stop=True)
                a_ps.append(ap_0)
                ap_1 = ps_work.tile([P, T], F32, name=f"aps_{layer}_1", tag="work")
                for j in range(2):
                    nc.tensor.matmul(
                        ap_1,
                        lhsT=attnT[:, P * (1 + j):P * (2 + j)],
                        rhs=v_bf[:, j, :],
                        start=(j == 0),
                        stop=(j == 1),
                    )
                a_ps.append(ap_1)
                lnA = pbig.tile([P, 2, T], BF16, name=f"lnA_{layer}", tag="lnA")
                _ln_pair(nc, pools, a_ps, lnA)
                lnAT = pbig.tile([P, 2, T], BF16, name=f"lnAT_{layer}", tag="lnAT")
                _transpose4(nc, pools, lnA, lnAT, ident_sb)

                # ---- y phase: y = relu(lnA @ Wy) * x;  yenc = y @ enc ----
                ye0 = ps_accum.tile([P, T], F32, name=f"ye0_{layer}", tag="acc0")
                ye1 = ps_accum.tile([P, T], F32, name=f"ye1_{layer}", tag="acc1")
                yenc = [ye0, ye1]
                for pc in range(NCH // 2):  # two n-chunks at a time
                    ch0y = 2 * pc
                    if ch0y % GX == 0:
                        wyg = pwy.tile([P, 2, GX * P], BF16,
                                       name=f"wyg_{layer}_{ch0y}", tag="wy")
                        for dk in range(2):
                            nc.sync.dma_start(
                                wyg[:, dk, :],
                                wy_d[P * dk:P * (dk + 1), P * ch0y:P * (ch0y + GX)],
                            )
                    y_pre = ps_work.tile([P, 2 * T], F32, name=f"ypre_{layer}_{pc}",
                                         tag="work")
                    for m in range(2):
                        co = P * ((ch0y + m) % GX)
                        for dk in range(2):
                            nc.tensor.matmul(
                                y_pre[:, T * m:T * (m + 1)],
                                lhsT=wyg[:, dk, co:co + P],
                                rhs=lnAT[:, dk, :],
                                start=(dk == 0),
                                stop=(dk == 1),
                            )
                    yr = py.tile([P, 2 * T], BF16, name=f"yr_{layer}_{pc}", tag="y")
                    nc.scalar.activation(yr, y_pre, ACT.Relu)
                    yt = py.tile([P, 2 * T], BF16, name=f"yt_{layer}_{pc}", tag="y")
                    nc.vector.tensor_mul(yt, yr, x_sb[:, ch0y:ch0y + 2, :])
                    for m in range(2):
                        for i in range(2):
                            nc.tensor.matmul(
                                yenc[i],
                                lhsT=yt[:, T * m + P * i:T * m + P * (i + 1)],
                                rhs=enc_sb[:, ch0y + m, :],
                                start=(ch0y + m == 0),
                                stop=(ch0y + m == NCH - 1),
                            )

                # ---- AllGather yenc partials (f32) + sum + LNs + residual ----
                ye_bnc = dram.tile([P, 2 * T], BF16, name=f"ye_bnc_{layer}",
                                   tag="ye_in")
                ye_sb = pexp.tile([P, 2 * T], BF16, name=f"ye_sb_{layer}",
                                  tag="yg", bufs=4)
                for i in range(2):
                    nc.scalar.copy(ye_sb[:, T * i:T * (i + 1)], yenc[i])
                nc.sync.dma_start(ye_bnc[:], ye_sb)
                ye_gth = dram.tile([P, 2 * T], BF16, name=f"ye_gth_{layer}",
                                   tag="ye_out")
                nc.gpsimd.collective_compute(
                    "AllReduce", ALU.add, replica_groups=groups,
                    ins=[ye_bnc.opt()], outs=[ye_gth.opt()],
                )
                ysum = pbig.tile([P, 2, T], BF16, name=f"ysum_{layer}", tag="ysum")
                nc.sync.dma_start(ysum.rearrange("p a t -> p (a t)"), ye_gth[:])
                lnY = pbig.tile([P, 2, T], F32, name=f"lnY_{layer}", tag="lnY")
                _ln_pair(nc, pools, [ysum[:, 0, :], ysum[:, 1, :]], lnY)
                vres = pbig.tile([P, 2, T], F32, name=f"vres_{layer}", tag="vres")
                for i in range(2):
                    nc.vector.tensor_add(vres[:, i, :], v[:, i, :], lnY[:, i, :])
                v = pv.tile([P, 2, T], F32, name=f"v_l{layer + 1}", tag="v")
                _ln_pair(nc, pools, [vres[:, 0, :], vres[:, 1, :]], v)

            # ---- readout: out = v @ ro  (vocab slice) ----
            v_bf = pbig.tile([P, 2, T], BF16, name="vbf_ro", tag="vbf")
            for i in range(2):
                nc.vector.tensor_copy(v_bf[:, i, :], v[:, i, :])
            vT = pbig.tile([P, 2, T], BF16, name="vT_ro", tag="vT")
            _transpose4(nc, pools, v_bf, vT, ident_sb)
            for c in range(NVCH):
                rog = pro.tile([P, 2, VCH], BF16, name=f"rog_{c}", tag="ro")
                for dk in range(2):
                    nc.sync.dma_start(
                        rog[:, dk, :],
                        ro_d[P * dk:P * (dk + 1), VCH * c:VCH * (c + 1)],
                    )
                for i in range(2):
                    lg = ps_work.tile([P, VCH], F32, name=f"lg_{c}_{i}", tag="work")
                    for dk in range(2):
                        nc.tensor.matmul(
                            lg,
                            lhsT=vT[:, dk, P * i:P * (i + 1)],
                            rhs=rog[:, dk, :],
                            start=(dk == 0),
                            stop=(dk == 1),
                        )
                    lg_sb = py.tile([P, VCH], F32, name=f"lg_sb_{c}_{i}",
                                    tag="lgsb", bufs=4)
                    nc.vector.tensor_copy(lg_sb, lg)
                    nc.sync.dma_start(
                        out_d[P * i:P * (i + 1), VCH * c:VCH * (c + 1)], lg_sb
                    )

    nc.compile()
    return nc


# ------------------------- host-side preparation -------------------------

def _pair_perm():
    """perm[new] = old index within a head, de-interleaving rope pairs."""
    perm = np.zeros(n_head, dtype=np.int64)
    for c in range(NPAIR):
        k = np.arange(P) + c * P          # pair indices in this pair-chunk
        perm[(2 * c) * P + np.arange(P)] = 2 * k
        perm[(2 * c + 1) * P + np.arange(P)] = 2 * k + 1
    return perm


def _rope_tables():
    """cs[p, c, 0:T] = cos, cs[p, c, T:2T] = sin, scaled by d**-0.25."""
    inv_freq = 1.0 / (
        ROPE_BASE ** (np.arange(0, n_head, 2, dtype=np.float32) / n_head)
    )  # (4096,) f32, matching reference arithmetic
    t = np.arange(T, dtype=np.float32)
    freqs = t[:, None] * inv_freq[None, :]         # (T, 4096) f32
    cos = np.cos(freqs) * S4                       # (T, 4096)
    sin = np.sin(freqs) * S4
    cs = np.zeros((P, NPAIR, 2 * T), dtype=np.float32)
    for c in range(NPAIR):
        k = c * P + np.arange(P)                   # (128,) pair indices
        cs[:, c, 0:T] = cos[:, k].T
        cs[:, c, T:2 * T] = sin[:, k].T
    return cs.astype(ml_dtypes.bfloat16)


def _masks():
    # [P, 2, T]: tile0 mask in [:, 0, 0:128] (s<=t); tile1 in [:, 1, 0:256]
    m = np.zeros((P, 2, T), dtype=np.float32)
    t = np.arange(P)[:, None]
    m[:, 0, 0:P] = (np.arange(P)[None, :] <= t).astype(np.float32)
    m[:, 1, :] = (np.arange(T)[None, :] <= t + P).astype(np.float32)
    return m.astype(ml_dtypes.bfloat16)


_CACHE = {}


def kernel(idx, wte, encoder, decoder_x, decoder_y, readout):
    if "nc" not in _CACHE:
        _CACHE["nc"] = build_nc()
    nc = _CACHE["nc"]
    in_maps = prepare_in_maps(idx, wte, encoder, decoder_x, decoder_y, readout)
    res = run_bass_kernel_spmd(nc, in_maps, core_ids=list(range(8)))
    return assemble_output([res.results[c]["out"] for c in range(8)])


def assemble_output(outs):
    out = np.empty((B, T, V), dtype=np.float32)
    for c in range(8):
        b, h = c // 4, c % 4
        out[b, :, h * VSLICE:(h + 1) * VSLICE] = outs[c]
    return out


def prepare_in_maps(idx, wte, encoder, decoder_x, decoder_y, readout):
    idx = np.asarray(idx)
    wte = np.ascontiguousarray(np.asarray(wte, dtype=np.float32))
    encoder = np.asarray(encoder, dtype=np.float32)
    decoder_x = np.asarray(decoder_x, dtype=np.float32)
    decoder_y = np.asarray(decoder_y, dtype=np.float32)
    readout = np.asarray(readout, dtype=np.float32)

    perm = _pair_perm()
    cs = _rope_tables()
    masks = _masks()
    ident = np.eye(P, dtype=np.float32).astype(ml_dtypes.bfloat16)
    bf = ml_dtypes.bfloat16

    wx_h = [np.ascontiguousarray(decoder_x[h][:, perm].astype(bf)) for h in range(H)]
    wy_h = [np.ascontiguousarray(decoder_y[h][:, perm].astype(bf)) for h in range(H)]
    enc_h = [
        np.ascontiguousarray(encoder[h * n_head + perm, :].astype(bf))
        for h in range(H)
    ]
    ro_h = [
        np.ascontiguousarray(readout[:, h * VSLICE:(h + 1) * VSLICE].astype(bf))
        for h in range(H)
    ]
    idx_b = [np.ascontiguousarray(idx[b].reshape(2, P).astype(np.int32))
             for b in range(B)]

    in_maps = []
    for c in range(8):
        b, h = c // 4, c % 4
        in_maps.append({
            "wte": wte,
            "idx2": idx_b[b],
            "wx": wx_h[h],
            "wy": wy_h[h],
            "enc": enc_h[h],
            "ro": ro_h[h],
            "cs": cs,
            "masks": masks,
            "ident": ident,
        })

    return in_maps


if __name__ == "__main__":
    nc = build_nc()
    print("built + compiled OK")


# revision 16
# speedup vs baseline: 1.1920x; 1.0036x over previous
"""Trainium2 Bass kernel for the BDH-style weight-tied transformer.

Contract: kernel(**inputs) takes FULL unsharded numpy inputs (idx, wte,
encoder, decoder_x, decoder_y, readout) and returns the FULL (B, T, V)
logits, running the model on 8 NeuronCores via run_bass_kernel_spmd.

Sharding: core c -> (b = c // 4, h = c % 4).  Group {0..3} handles batch 0,
{4..7} batch 1.  Within a group: tensor-parallel over heads with
AllGather + local-sum for (a) the head-summed attention matrix and (b) the
y @ encoder projection.  LayerNorm is scale-invariant, so summing heads
(instead of averaging) is exact.  Readout is vocab-split 4 ways per group.

The neuron axis of each head is permuted host-side so RoPE pair partners
(2k, 2k+1) live at the same partition of sibling 128-chunks ("even" chunk
2c / "odd" chunk 2c+1).  The rotation then needs no cross-partition data
movement.  The 1/sqrt(d) attention scale is folded into the cos/sin tables
(d**-0.25 on each factor of the Gram matrix).
"""

import sys

for _p in ("/opt/trn_rl_repo", "/opt/pypackages"):
    if _p not in sys.path:
        sys.path.append(_p)

import ml_dtypes
import numpy as np

import concourse.bass as bass
import concourse.mybir as mybir
import concourse.tile as tile
from concourse import bacc
from concourse.bass_utils import run_bass_kernel_spmd

F32 = mybir.dt.float32
BF16 = mybir.dt.bfloat16
I32 = mybir.dt.int32
AX = mybir.AxisListType
ALU = mybir.AluOpType
ACT = mybir.ActivationFunctionType

# Model dims (hardcoded per problem spec)
B, T, D, H, N, V = 2, 256, 256, 4, 32768, 32000
n_head = N // H            # 8192 neurons per head (one core's slice)
P = 128
NCH = n_head // P          # 64 chunks of 128 neurons
NPAIR = NCH // 2           # 32 pair-chunks
L_LAYERS = 6
LN_EPS = 1e-5
ROPE_BASE = 10000.0
VSLICE = V // 4            # 8000 vocab columns per core
VCH = 500                  # vocab chunk (PSUM bank holds 512 f32)
NVCH = VSLICE // VCH       # 16
GX = 4                     # n-chunks per streamed weight group
S4 = float(n_head) ** -0.25


def _ln_pair(nc, pools, srcs, out, out_dtype_note=""):
    """LayerNorm over the free dim (D=256) of two [128, 256] f32 tiles.

    srcs: list of 2 APs (SBUF or PSUM, f32).  out: [128, 2, 256] tile.
    """
    psmall = pools["small"]
    for i, src in enumerate(srcs):
        stats = psmall.tile([P, 6], F32, name=f"ln_st{i}", tag="lnstat")
        nc.vector.bn_stats(stats, src)
        aggr = psmall.tile([P, 2], F32, name=f"ln_ag{i}", tag="lnstat")
        nc.vector.bn_aggr(aggr, stats)
        std = psmall.tile([P, 1], F32, name=f"ln_std{i}", tag="lnstat")
        nc.scalar.activation(std, aggr[:, 1:2], ACT.Sqrt,
                             bias=pools["eps"][:, :1])
        rinv = psmall.tile([P, 1], F32, name=f"ln_rinv{i}", tag="lnstat")
        nc.vector.reciprocal(rinv, std)
        nc.vector.tensor_scalar(out[:, i, :], src, aggr[:, 0:1], rinv,
                                op0=ALU.subtract, op1=ALU.mult)


def _transpose4(nc, pools, src, dst, ident):
    """dst[:, k, 128*i:128*(i+1)] = src[:, i, 128*k:128*(k+1)].T  (bf16).

    src, dst: [128, 2, 256] bf16.  Four PE transposes + ACT copies.
    """
    pwork = pools["ps_work"]
    for i in range(2):
        for k in range(2):
            tp = pwork.tile([P, P], BF16, name=f"tp_{i}_{k}", tag="work")
            nc.tensor.transpose(tp, src[:, i, P * k:P * (k + 1)], ident)
            nc.scalar.copy(dst[:, k, P * i:P * (i + 1)], tp)


def build_nc(num_cores=8):
    nc = bacc.Bacc(
        "TRN2", target_bir_lowering=False, debug=False, num_devices=num_cores
    )

    # ---- DRAM I/O (per-core data supplied via in_maps) ----
    wte_d = nc.dram_tensor("wte", [V, D], F32, kind="ExternalInput").ap()
    idx_d = nc.dram_tensor("idx2", [2, P], I32, kind="ExternalInput").ap()
    wx_d = nc.dram_tensor("wx", [D, n_head], BF16, kind="ExternalInput").ap()
    wy_d = nc.dram_tensor("wy", [D, n_head], BF16, kind="ExternalInput").ap()
    enc_d = nc.dram_tensor("enc", [n_head, D], BF16, kind="ExternalInput").ap()
    ro_d = nc.dram_tensor("ro", [D, VSLICE], BF16, kind="ExternalInput").ap()
    cs_d = nc.dram_tensor("cs", [P, NPAIR, 2 * T], BF16, kind="ExternalInput").ap()
    masks_d = nc.dram_tensor("masks", [P, 2, T], BF16, kind="ExternalInput").ap()
    ident_d = nc.dram_tensor("ident", [P, P], BF16, kind="ExternalInput").ap()
    out_d = nc.dram_tensor("out", [T, VSLICE], F32, kind="ExternalOutput").ap()

    groups = [[0, 1, 2, 3], [4, 5, 6, 7]]

    with tile.TileContext(nc) as tc:
        with (
            tc.tile_pool(name="pers", bufs=1) as pers,
            tc.tile_pool(name="pv", bufs=2) as pv,
            tc.tile_pool(name="pbig", bufs=2) as pbig,
            tc.tile_pool(name="pwx", bufs=3) as pwx,
            tc.tile_pool(name="pwy", bufs=3) as pwy,
            tc.tile_pool(name="pro", bufs=3) as pro,
            tc.tile_pool(name="pxr", bufs=8) as pxr,
            tc.tile_pool(name="py", bufs=6) as py,
            tc.tile_pool(name="psmall", bufs=12) as psmall,
            tc.tile_pool(name="pcent", bufs=4) as pcent,
            tc.tile_pool(name="psq", bufs=2) as psq,
            tc.tile_pool(name="pexp", bufs=8) as pexp,
            tc.tile_pool(name="ps_work", bufs=5, space="PSUM") as ps_work,
            tc.tile_pool(name="ps_accum", bufs=1, space="PSUM") as ps_accum,
            tc.tile_pool(name="dram", bufs=2, space="DRAM") as dram,
        ):
            pools = {
                "small": psmall,
                "cent": pcent,
                "sq": psq,
                "ps_work": ps_work,
            }

            # ---- persistent SBUF tensors ----
            eps_sb = pers.tile([P, 1], F32, name="eps_sb", tag="eps")
            nc.vector.memset(eps_sb, LN_EPS)
            pools["eps"] = eps_sb
            cs_sb = pers.tile([P, NPAIR, 2 * T], BF16, name="cs_sb", tag="cs")
            masks_sb = pers.tile([P, 2, T], BF16, name="masks_sb", tag="masks")
            ident_sb = pers.tile([P, P], BF16, name="ident_sb", tag="ident")
            enc_sb = pers.tile([P, NCH, T], BF16, name="enc_sb", tag="enc")
            x_sb = pers.tile([P, NCH, T], BF16, name="x_sb", tag="x")

            for g in range(4):
                nc.sync.dma_start(
                    cs_sb[:, 8 * g:8 * (g + 1), :], cs_d[:, 8 * g:8 * (g + 1), :]
                )
            nc.sync.dma_start(masks_sb[:], masks_d[:])
            nc.sync.dma_start(ident_sb[:], ident_d[:])
            enc_r = enc_d.rearrange("(c p) d -> p c d", p=P)
            for g in range(8):
                nc.sync.dma_start(
                    enc_sb[:, 8 * g:8 * (g + 1), :], enc_r[:, 8 * g:8 * (g + 1), :]
                )

            # ---- embedding gather + first LN ----
            vraw = pbig.tile([P, 2, T], F32, name="vraw", tag="vraw")
            for i in range(2):
                idx_sb = psmall.tile([P, 1], I32, name=f"idx_sb{i}", tag="idx")
                nc.sync.dma_start(idx_sb, idx_d[i, :].rearrange("(p o) -> p o", o=1))
                nc.gpsimd.indirect_dma_start(
                    out=vraw[:, i, :],
                    out_offset=None,
                    in_=wte_d[:],
                    in_offset=bass.IndirectOffsetOnAxis(ap=idx_sb[:, :1], axis=0),
                )
            v = pv.tile([P, 2, T], F32, name="v_l0", tag="v")
            _ln_pair(nc, pools, [vraw[:, 0, :], vraw[:, 1, :]], v)

            for layer in range(L_LAYERS):
                # ---- v_bf (natural, bf16) and vT (transposed, bf16) ----
                v_bf = pbig.tile([P, 2, T], BF16, name=f"vbf_{layer}", tag="vbf")
                for i in range(2):
                    nc.vector.tensor_copy(v_bf[:, i, :], v[:, i, :])
                vT = pbig.tile([P, 2, T], BF16, name=f"vT_{layer}", tag="vT")
                _transpose4(nc, pools, v_bf, vT, ident_sb)

                # ---- x phase: x = relu(v @ Wx), rope, scores (Gram) ----
                sc0 = ps_accum.tile([P, P], F32, name=f"sc0_{layer}", tag="acc0")
                sc1 = ps_accum.tile([P, T], F32, name=f"sc1_{layer}", tag="acc1")
                scores = [sc0, sc1]
                def emit_scores(grp, xr_e, xr_o):
                    ch0 = 4 * grp
                    for q in range(2):  # pair-chunk within group
                        for xr in (xr_e, xr_o):
                            chv = ch0 + 2 * q + (0 if xr is xr_e else 1)
                            nc.tensor.matmul(
                                scores[0],
                                lhsT=xr[:, q, 0:P],
                                rhs=xr[:, q, 0:P],
                                start=(chv == 0),
                                stop=(chv == NCH - 1),
                            )
                            nc.tensor.matmul(
                                scores[1],
                                lhsT=xr[:, q, P:2 * P],
                                rhs=xr[:, q, :],
                                start=(chv == 0),
                                stop=(chv == NCH - 1),
                            )

                pending = None  # (grp, xr_e, xr_o) awaiting scores emission
                for grp in range(NPAIR // 2):  # 16 groups of 2 pair-chunks
                    ch0 = 4 * grp  # first of 4 n-chunks in this group
                    if ch0 % GX == 0:
                        wxg = pwx.tile([P, 2, GX * P], BF16,
                                       name=f"wxg_{layer}_{ch0}", tag="wx")
                        for dk in range(2):
                            nc.sync.dma_start(
                                wxg[:, dk, :],
                                wx_d[P * dk:P * (dk + 1),
                                     P * ch0:P * (ch0 + GX)],
                            )
                    for pc in (2 * grp, 2 * grp + 1):
                        x_pre = ps_work.tile([P, 2 * T], F32,
                                             name=f"xpre_{layer}_{pc}", tag="work")
                        for m in range(2):  # even / odd member chunk
                            ch = 2 * pc + m
                            co = P * (ch % GX)
                            for dk in range(2):
                                nc.tensor.matmul(
                                    x_pre[:, T * m:T * (m + 1)],
                                    lhsT=wxg[:, dk, co:co + P],
                                    rhs=vT[:, dk, :],
                                    start=(dk == 0),
                                    stop=(dk == 1),
                                )
                        nc.scalar.activation(
                            x_sb[:, 2 * pc:2 * pc + 2, :], x_pre, ACT.Relu)
                    # rope over the 2 pair-chunks (even chunks ch0, ch0+2;
                    # odd chunks ch0+1, ch0+3), batched FD=512
                    xe = x_sb[:, ch0:ch0 + 4:2, :]
                    xo = x_sb[:, ch0 + 1:ch0 + 4:2, :]
                    cvw = cs_sb[:, 2 * grp:2 * grp + 2, 0:T]
                    svw = cs_sb[:, 2 * grp:2 * grp + 2, T:2 * T]
                    m_ec = pxr.tile([P, 2, T], BF16, name=f"mec_{layer}_{grp}", tag="xr", bufs=12)
                    m_os = pxr.tile([P, 2, T], BF16, name=f"mos_{layer}_{grp}", tag="xr", bufs=12)
                    m_oc = pxr.tile([P, 2, T], BF16, name=f"moc_{layer}_{grp}", tag="xr", bufs=12)
                    m_es = pxr.tile([P, 2, T], BF16, name=f"mes_{layer}_{grp}", tag="xr", bufs=12)
                    xr_e = pxr.tile([P, 2, T], BF16, name=f"xre_{layer}_{grp}", tag="xr", bufs=12)
                    xr_o = pxr.tile([P, 2, T], BF16, name=f"xro_{layer}_{grp}", tag="xr", bufs=12)
                    nc.vector.tensor_mul(m_ec, xe, cvw)
                    nc.vector.tensor_mul(m_os, xo, svw)
                    nc.vector.tensor_sub(xr_e, m_ec, m_os)
                    nc.vector.tensor_mul(m_oc, xo, cvw)
                    nc.vector.tensor_mul(m_es, xe, svw)
                    nc.vector.tensor_add(xr_o, m_oc, m_es)
                    if pending is not None:
                        emit_scores(*pending)
                    pending = (grp, xr_e, xr_o)
                emit_scores(*pending)

                # ---- softmax (causal, per-head normalized) ----
                # attn packed [128, 384]: cols 0:128 = t-tile0 (s<128),
                # cols 128:384 = t-tile1 (s<256)
                attn = pexp.tile([P, 3 * P], BF16, name=f"attn_{layer}", tag="attn", bufs=2)
                for i, (w, lo) in enumerate(((P, 0), (T, P))):
                    mx = psmall.tile([P, 1], F32, name=f"mx_{i}", tag="lnstat")
                    nc.vector.tensor_reduce(mx, scores[i], axis=AX.X, op=ALU.max)
                    negmx = psmall.tile([P, 1], F32, name=f"negmx_{i}", tag="lnstat")
                    nc.vector.tensor_scalar_mul(negmx, mx, -1.0)
                    ex = pexp.tile([P, w], BF16, name=f"ex_{layer}_{i}", tag="ex", bufs=2)
                    nc.scalar.activation(ex, scores[i], ACT.Exp, bias=negmx)
                    nc.vector.tensor_mul(ex, ex, masks_sb[:, i, 0:w])
                    rs = psmall.tile([P, 1], F32, name=f"rs_{i}", tag="lnstat")
                    nc.vector.tensor_reduce(rs, ex, axis=AX.X, op=ALU.add)
                    rcp = psmall.tile([P, 1], F32, name=f"rcp_{i}", tag="lnstat")
                    nc.vector.reciprocal(rcp, rs)
                    nc.vector.tensor_scalar_mul(attn[:, lo:lo + w], ex, rcp)

                # ---- AllGather attn over the 4-core group; sum heads ----
                attn_bnc = dram.tile([P, 3 * P], BF16,
                                     name=f"attn_bnc_{layer}", tag="attn_in")
                nc.gpsimd.dma_start(attn_bnc[:], attn[:])
                attn_gth = dram.tile([P, 3 * P], BF16, name=f"attn_gth_{layer}",
                                     tag="attn_out")
                nc.gpsimd.collective_compute(
                    "AllReduce", ALU.add, replica_groups=groups,
                    ins=[attn_bnc.opt()], outs=[attn_gth.opt()],
                )
                asum = pexp.tile([P, 3 * P], BF16, name=f"asum_{layer}", tag="asum", bufs=2)
                nc.sync.dma_start(asum, attn_gth[:])

                # ---- transpose summed attn; a = attnT.T @ v; LN(a) ----
                # attnT blocks: b00 = attn[t0, s0].T; b10/b11 = attn[t1, :].T
                attnT = pexp.tile([P, 3 * P], BF16, name=f"attnT_{layer}", tag="attnT", bufs=2)
                for bi, (alo, tlo) in enumerate(((0, 0), (P, P), (2 * P, 2 * P))):
                    tp = ps_work.tile([P, P], BF16, name=f"tpa_{bi}", tag="work")
                    nc.tensor.transpose(tp, asum[:, alo:alo + P], ident_sb)
                    nc.scalar.copy(attnT[:, tlo:tlo + P], tp)
                a_ps = []
                ap_0 = ps_work.tile([P, T], F32, name=f"aps_{layer}_0", tag="work")
                nc.tensor.matmul(ap_0, lhsT=attnT[:, 0:P], rhs=v_bf[:, 0, :],
                                 start=True, stop=True)
                a_ps.append(ap_0)
                ap_1 = ps_work.tile([P, T], F32, name=f"aps_{layer}_1", tag="work")
                for j in range(2):
                    nc.tensor.matmul(
                        ap_1,
                        lhsT=attnT[:, P * (1 + j):P * (2 + j)],
                        rhs=v_bf[:, j, :],
                        start=(j == 0),
                        stop=(j == 1),
                    )
                a_ps.append(ap_1)
                lnA = pbig.tile([P, 2, T], BF16, name=f"lnA_{layer}", tag="lnA")
                _ln_pair(nc, pools, a_ps, lnA)
                lnAT = pbig.tile([P, 2, T], BF16, name=f"lnAT_{layer}", tag="lnAT")
                _transpose4(nc, pools, lnA, lnAT, ident_sb)

                # ---- y phase: y = relu(lnA @ Wy) * x;  yenc = y @ enc ----
                ye0 = ps_accum.tile([P, T], F32, name=f"ye0_{layer}", tag="acc0")
                ye1 = ps_accum.tile([P, T], F32, name=f"ye1_{layer}", tag="acc1")
                yenc = [ye0, ye1]
                def emit_yenc(pc, yt):
                    ch0y = 2 * pc
                    for m in range(2):
                        for i in range(2):
                            nc.tensor.matmul(
                                yenc[i],
                                lhsT=yt[:, T * m + P * i:T * m + P * (i + 1)],
                                rhs=enc_sb[:, ch0y + m, :],
                                start=(ch0y + m == 0),
                                stop=(ch0y + m == NCH - 1),
                            )

                pend_y = None
                for pc in range(NCH // 2):  # two n-chunks at a time
                    ch0y = 2 * pc
                    if ch0y % GX == 0:
                        wyg = pwy.tile([P, 2, GX * P], BF16,
                                       name=f"wyg_{layer}_{ch0y}", tag="wy")
                        for dk in range(2):
                            nc.sync.dma_start(
                                wyg[:, dk, :],
                                wy_d[P * dk:P * (dk + 1), P * ch0y:P * (ch0y + GX)],
                            )
                    y_pre = ps_work.tile([P, 2 * T], F32, name=f"ypre_{layer}_{pc}",
                                         tag="work")
                    for m in range(2):
                        co = P * ((ch0y + m) % GX)
                        for dk in range(2):
                            nc.tensor.matmul(
                                y_pre[:, T * m:T * (m + 1)],
                                lhsT=wyg[:, dk, co:co + P],
                                rhs=lnAT[:, dk, :],
                                start=(dk == 0),
                                stop=(dk == 1),
                            )
                    yr = py.tile([P, 2 * T], BF16, name=f"yr_{layer}_{pc}", tag="y", bufs=8)
                    nc.scalar.activation(yr, y_pre, ACT.Relu)
                    yt = py.tile([P, 2 * T], BF16, name=f"yt_{layer}_{pc}", tag="y", bufs=8)
                    nc.vector.tensor_mul(yt, yr, x_sb[:, ch0y:ch0y + 2, :])
                    if pend_y is not None:
                        emit_yenc(*pend_y)
                    pend_y = (pc, yt)
                emit_yenc(*pend_y)

                # ---- AllGather yenc partials (f32) + sum + LNs + residual ----
                ye_bnc = dram.tile([P, 2 * T], BF16, name=f"ye_bnc_{layer}",
                                   tag="ye_in")
                ye_sb = pexp.tile([P, 2 * T], BF16, name=f"ye_sb_{layer}",
                                  tag="yg", bufs=4)
                for i in range(2):
                    nc.scalar.copy(ye_sb[:, T * i:T * (i + 1)], yenc[i])
                nc.sync.dma_start(ye_bnc[:], ye_sb)
                ye_gth = dram.tile([P, 2 * T], BF16, name=f"ye_gth_{layer}",
                                   tag="ye_out")
                nc.gpsimd.collective_compute(
                    "AllReduce", ALU.add, replica_groups=groups,
                    ins=[ye_bnc.opt()], outs=[ye_gth.opt()],
                )
                ysum = pbig.tile([P, 2, T], BF16, name=f"ysum_{layer}", tag="ysum")
                nc.sync.dma_start(ysum.rearrange("p a t -> p (a t)"), ye_gth[:])
                lnY = pbig.tile([P, 2, T], F32, name=f"lnY_{layer}", tag="lnY")
                _ln_pair(nc, pools, [ysum[:, 0, :], ysum[:, 1, :]], lnY)
                vres = pbig.tile([P, 2, T], F32, name=f"vres_{layer}", tag="vres")
                for i in range(2):
                    nc.vector.tensor_add(vres[:, i, :], v[:, i, :], lnY[:, i, :])
                v = pv.tile([P, 2, T], F32, name=f"v_l{layer + 1}", tag="v")
                _ln_pair(nc, pools, [vres[:, 0, :], vres[:, 1, :]], v)

            # ---- readout: out = v @ ro  (vocab slice) ----
            v_bf = pbig.tile([P, 2, T], BF16, name="vbf_ro", tag="vbf")
            for i in range(2):
                nc.vector.tensor_copy(v_bf[:, i, :], v[:, i, :])
            vT = pbig.tile([P, 2, T], BF16, name="vT_ro", tag="vT")
            _transpose4(nc, pools, v_bf, vT, ident_sb)
            for c in range(NVCH):
                rog = pro.tile([P, 2, VCH], BF16, name=f"rog_{c}", tag="ro")
                for dk in range(2):
                    nc.sync.dma_start(
                        rog[:, dk, :],
                        ro_d[P * dk:P * (dk + 1), VCH * c:VCH * (c + 1)],
                    )
                for i in range(2):
                    lg = ps_work.tile([P, VCH], F32, name=f"lg_{c}_{i}", tag="work")
                    for dk in range(2):
                        nc.tensor.matmul(
                            lg,
                            lhsT=vT[:, dk, P * i:P * (i + 1)],
                            rhs=rog[:, dk, :],
                            start=(dk == 0),
                            stop=(dk == 1),
                        )
                    lg_sb = py.tile([P, VCH], F32, name=f"lg_sb_{c}_{i}",
                                    tag="lgsb", bufs=4)
                    nc.vector.tensor_copy(lg_sb, lg)
                    nc.sync.dma_start(
                        out_d[P * i:P * (i + 1), VCH * c:VCH * (c + 1)], lg_sb
                    )

    nc.compile()
    return nc


# ------------------------- host-side preparation -------------------------

def _pair_perm():
    """perm[new] = old index within a head, de-interleaving rope pairs."""
    perm = np.zeros(n_head, dtype=np.int64)
    for c in range(NPAIR):
        k = np.arange(P) + c * P          # pair indices in this pair-chunk
        perm[(2 * c) * P + np.arange(P)] = 2 * k
        perm[(2 * c + 1) * P + np.arange(P)] = 2 * k + 1
    return perm


def _rope_tables():
    """cs[p, c, 0:T] = cos, cs[p, c, T:2T] = sin, scaled by d**-0.25."""
    inv_freq = 1.0 / (
        ROPE_BASE ** (np.arange(0, n_head, 2, dtype=np.float32) / n_head)
    )  # (4096,) f32, matching reference arithmetic
    t = np.arange(T, dtype=np.float32)
    freqs = t[:, None] * inv_freq[None, :]         # (T, 4096) f32
    cos = np.cos(freqs) * S4                       # (T, 4096)
    sin = np.sin(freqs) * S4
    cs = np.zeros((P, NPAIR, 2 * T), dtype=np.float32)
    for c in range(NPAIR):
        k = c * P + np.arange(P)                   # (128,) pair indices
        cs[:, c, 0:T] = cos[:, k].T
        cs[:, c, T:2 * T] = sin[:, k].T
    return cs.astype(ml_dtypes.bfloat16)


def _masks():
    # [P, 2, T]: tile0 mask in [:, 0, 0:128] (s<=t); tile1 in [:, 1, 0:256]
    m = np.zeros((P, 2, T), dtype=np.float32)
    t = np.arange(P)[:, None]
    m[:, 0, 0:P] = (np.arange(P)[None, :] <= t).astype(np.float32)
    m[:, 1, :] = (np.arange(T)[None, :] <= t + P).astype(np.float32)
    return m.astype(ml_dtypes.bfloat16)


_CACHE = {}


def kernel(idx, wte, encoder, decoder_x, decoder_y, readout):
    if "nc" not in _CACHE:
        _CACHE["nc"] = build_nc()
    nc = _CACHE["nc"]
    in_maps = prepare_in_maps(idx, wte, encoder, decoder_x, decoder_y, readout)
    res = run_bass_kernel_spmd(nc, in_maps, core_ids=list(range(8)))
    return assemble_output([res.results[c]["out"] for c in range(8)])


def assemble_output(outs):
    out = np.empty((B, T, V), dtype=np.float32)
    for c in range(8):
        b, h = c // 4, c % 4
        out[b, :, h * VSLICE:(h + 1) * VSLICE] = outs[c]
    return out


def prepare_in_maps(idx, wte, encoder, decoder_x, decoder_y, readout):
    idx = np.asarray(idx)
    wte = np.ascontiguousarray(np.asarray(wte, dtype=np.float32))
    encoder = np.asarray(encoder, dtype=np.float32)
    decoder_x = np.asarray(decoder_x, dtype=np.float32)
    decoder_y = np.asarray(decoder_y, dtype=np.float32)
    readout = np.asarray(readout, dtype=np.float32)

    perm = _pair_perm()
    cs = _rope_tables()
    masks = _masks()
    ident = np.eye(P, dtype=np.float32).astype(ml_dtypes.bfloat16)
    bf = ml_dtypes.bfloat16

    wx_h = [np.ascontiguousarray(decoder_x[h][:, perm].astype(bf)) for h in range(H)]
    wy_h = [np.ascontiguousarray(decoder_y[h][:, perm].astype(bf)) for h in range(H)]
    enc_h = [
        np.ascontiguousarray(encoder[h * n_head + perm, :].astype(bf))
        for h in range(H)
    ]
    ro_h = [
        np.ascontiguousarray(readout[:, h * VSLICE:(h + 1) * VSLICE].astype(bf))
        for h in range(H)
    ]
    idx_b = [np.ascontiguousarray(idx[b].reshape(2, P).astype(np.int32))
             for b in range(B)]

    in_maps = []
    for c in range(8):
        b, h = c // 4, c % 4
        in_maps.append({
            "wte": wte,
            "idx2": idx_b[b],
            "wx": wx_h[h],
            "wy": wy_h[h],
            "enc": enc_h[h],
            "ro": ro_h[h],
            "cs": cs,
            "masks": masks,
            "ident": ident,
        })

    return in_maps


if __name__ == "__main__":
    nc = build_nc()
    print("built + compiled OK")


# revision 17
# speedup vs baseline: 1.2005x; 1.0071x over previous
"""Trainium2 Bass kernel for the BDH-style weight-tied transformer.

Contract: kernel(**inputs) takes FULL unsharded numpy inputs (idx, wte,
encoder, decoder_x, decoder_y, readout) and returns the FULL (B, T, V)
logits, running the model on 8 NeuronCores via run_bass_kernel_spmd.

Sharding: core c -> (b = c // 4, h = c % 4).  Group {0..3} handles batch 0,
{4..7} batch 1.  Within a group: tensor-parallel over heads with
AllGather + local-sum for (a) the head-summed attention matrix and (b) the
y @ encoder projection.  LayerNorm is scale-invariant, so summing heads
(instead of averaging) is exact.  Readout is vocab-split 4 ways per group.

The neuron axis of each head is permuted host-side so RoPE pair partners
(2k, 2k+1) live at the same partition of sibling 128-chunks ("even" chunk
2c / "odd" chunk 2c+1).  The rotation then needs no cross-partition data
movement.  The 1/sqrt(d) attention scale is folded into the cos/sin tables
(d**-0.25 on each factor of the Gram matrix).
"""

import sys

for _p in ("/opt/trn_rl_repo", "/opt/pypackages"):
    if _p not in sys.path:
        sys.path.append(_p)

import ml_dtypes
import numpy as np

import concourse.bass as bass
import concourse.mybir as mybir
import concourse.tile as tile
from concourse import bacc
from concourse.bass_utils import run_bass_kernel_spmd

F32 = mybir.dt.float32
BF16 = mybir.dt.bfloat16
I32 = mybir.dt.int32
AX = mybir.AxisListType
ALU = mybir.AluOpType
ACT = mybir.ActivationFunctionType

# Model dims (hardcoded per problem spec)
B, T, D, H, N, V = 2, 256, 256, 4, 32768, 32000
n_head = N // H            # 8192 neurons per head (one core's slice)
P = 128
NCH = n_head // P          # 64 chunks of 128 neurons
NPAIR = NCH // 2           # 32 pair-chunks
L_LAYERS = 6
LN_EPS = 1e-5
ROPE_BASE = 10000.0
VSLICE = V // 4            # 8000 vocab columns per core
VCH = 500                  # vocab chunk (PSUM bank holds 512 f32)
NVCH = VSLICE // VCH       # 16
GX = 4                     # n-chunks per streamed weight group
S4 = float(n_head) ** -0.25


def _ln_pair(nc, pools, srcs, out, out_dtype_note=""):
    """LayerNorm over the free dim (D=256) of two [128, 256] f32 tiles.

    srcs: list of 2 APs (SBUF or PSUM, f32).  out: [128, 2, 256] tile.
    """
    psmall = pools["small"]
    for i, src in enumerate(srcs):
        stats = psmall.tile([P, 6], F32, name=f"ln_st{i}", tag="lnstat")
        nc.vector.bn_stats(stats, src)
        aggr = psmall.tile([P, 2], F32, name=f"ln_ag{i}", tag="lnstat")
        nc.vector.bn_aggr(aggr, stats)
        std = psmall.tile([P, 1], F32, name=f"ln_std{i}", tag="lnstat")
        nc.scalar.activation(std, aggr[:, 1:2], ACT.Sqrt,
                             bias=pools["eps"][:, :1])
        rinv = psmall.tile([P, 1], F32, name=f"ln_rinv{i}", tag="lnstat")
        nc.vector.reciprocal(rinv, std)
        nc.vector.tensor_scalar(out[:, i, :], src, aggr[:, 0:1], rinv,
                                op0=ALU.subtract, op1=ALU.mult)


def _transpose4(nc, pools, src, dst, ident):
    """dst[:, k, 128*i:128*(i+1)] = src[:, i, 128*k:128*(k+1)].T  (bf16).

    src, dst: [128, 2, 256] bf16.  Four PE transposes + ACT copies.
    """
    pwork = pools["ps_work"]
    for i in range(2):
        for k in range(2):
            tp = pwork.tile([P, P], BF16, name=f"tp_{i}_{k}", tag="work")
            nc.tensor.transpose(tp, src[:, i, P * k:P * (k + 1)], ident)
            nc.scalar.copy(dst[:, k, P * i:P * (i + 1)], tp)


def build_nc(num_cores=8):
    nc = bacc.Bacc(
        "TRN2", target_bir_lowering=False, debug=False, num_devices=num_cores
    )

    # ---- DRAM I/O (per-core data supplied via in_maps) ----
    wte_d = nc.dram_tensor("wte", [V, D], F32, kind="ExternalInput").ap()
    idx_d = nc.dram_tensor("idx2", [2, P], I32, kind="ExternalInput").ap()
    wx_d = nc.dram_tensor("wx", [D, n_head], BF16, kind="ExternalInput").ap()
    wy_d = nc.dram_tensor("wy", [D, n_head], BF16, kind="ExternalInput").ap()
    enc_d = nc.dram_tensor("enc", [n_head, D], BF16, kind="ExternalInput").ap()
    ro_d = nc.dram_tensor("ro", [D, VSLICE], BF16, kind="ExternalInput").ap()
    cs_d = nc.dram_tensor("cs", [P, NPAIR, 2 * T], BF16, kind="ExternalInput").ap()
    masks_d = nc.dram_tensor("masks", [P, 2, T], BF16, kind="ExternalInput").ap()
    ident_d = nc.dram_tensor("ident", [P, P], BF16, kind="ExternalInput").ap()
    out_d = nc.dram_tensor("out", [T, VSLICE], F32, kind="ExternalOutput").ap()

    groups = [[0, 1, 2, 3], [4, 5, 6, 7]]

    with tile.TileContext(nc) as tc:
        with (
            tc.tile_pool(name="pers", bufs=1) as pers,
            tc.tile_pool(name="pv", bufs=2) as pv,
            tc.tile_pool(name="pbig", bufs=2) as pbig,
            tc.tile_pool(name="pwx", bufs=4) as pwx,
            tc.tile_pool(name="pwy", bufs=4) as pwy,
            tc.tile_pool(name="pro", bufs=3) as pro,
            tc.tile_pool(name="pxr", bufs=8) as pxr,
            tc.tile_pool(name="py", bufs=6) as py,
            tc.tile_pool(name="psmall", bufs=16) as psmall,
            tc.tile_pool(name="pcent", bufs=4) as pcent,
            tc.tile_pool(name="psq", bufs=2) as psq,
            tc.tile_pool(name="pexp", bufs=8) as pexp,
            tc.tile_pool(name="ps_work", bufs=5, space="PSUM") as ps_work,
            tc.tile_pool(name="ps_accum", bufs=1, space="PSUM") as ps_accum,
            tc.tile_pool(name="dram", bufs=2, space="DRAM") as dram,
        ):
            pools = {
                "small": psmall,
                "cent": pcent,
                "sq": psq,
                "ps_work": ps_work,
            }

            # ---- persistent SBUF tensors ----
            eps_sb = pers.tile([P, 1], F32, name="eps_sb", tag="eps")
            nc.vector.memset(eps_sb, LN_EPS)
            pools["eps"] = eps_sb
            cs_sb = pers.tile([P, NPAIR, 2 * T], BF16, name="cs_sb", tag="cs")
            masks_sb = pers.tile([P, 2, T], BF16, name="masks_sb", tag="masks")
            ident_sb = pers.tile([P, P], BF16, name="ident_sb", tag="ident")
            enc_sb = pers.tile([P, NCH, T], BF16, name="enc_sb", tag="enc")
            x_sb = pers.tile([P, NCH, T], BF16, name="x_sb", tag="x")

            for g in range(4):
                nc.sync.dma_start(
                    cs_sb[:, 8 * g:8 * (g + 1), :], cs_d[:, 8 * g:8 * (g + 1), :]
                )
            nc.sync.dma_start(masks_sb[:], masks_d[:])
            nc.sync.dma_start(ident_sb[:], ident_d[:])
            enc_r = enc_d.rearrange("(c p) d -> p c d", p=P)
            for g in range(8):
                nc.sync.dma_start(
                    enc_sb[:, 8 * g:8 * (g + 1), :], enc_r[:, 8 * g:8 * (g + 1), :]
                )

            # ---- warm-up collective (first CC call pays ~40us extra) ----
            warm_in = dram.tile([P, 8], F32, name="warm_in", tag="warm_in")
            warm_sb = pexp.tile([P, 8], F32, name="warm_sb", tag="warm", bufs=1)
            nc.vector.memset(warm_sb, 0.0)
            nc.gpsimd.dma_start(warm_in[:], warm_sb)
            warm_out = dram.tile([P, 8], F32, name="warm_out", tag="warm_out")
            nc.gpsimd.collective_compute(
                "AllReduce", ALU.add, replica_groups=groups,
                ins=[warm_in.opt()], outs=[warm_out.opt()],
            )

            # ---- embedding gather + first LN ----
            vraw = pbig.tile([P, 2, T], F32, name="vraw", tag="vraw")
            for i in range(2):
                idx_sb = psmall.tile([P, 1], I32, name=f"idx_sb{i}", tag="idx")
                nc.sync.dma_start(idx_sb, idx_d[i, :].rearrange("(p o) -> p o", o=1))
                nc.gpsimd.indirect_dma_start(
                    out=vraw[:, i, :],
                    out_offset=None,
                    in_=wte_d[:],
                    in_offset=bass.IndirectOffsetOnAxis(ap=idx_sb[:, :1], axis=0),
                )
            v = pv.tile([P, 2, T], F32, name="v_l0", tag="v")
            _ln_pair(nc, pools, [vraw[:, 0, :], vraw[:, 1, :]], v)

            for layer in range(L_LAYERS):
                # ---- v_bf (natural, bf16) and vT (transposed, bf16) ----
                v_bf = pbig.tile([P, 2, T], BF16, name=f"vbf_{layer}", tag="vbf")
                for i in range(2):
                    nc.vector.tensor_copy(v_bf[:, i, :], v[:, i, :])
                vT = pbig.tile([P, 2, T], BF16, name=f"vT_{layer}", tag="vT")
                _transpose4(nc, pools, v_bf, vT, ident_sb)

                # ---- x phase: x = relu(v @ Wx), rope, scores (Gram) ----
                sc0 = ps_accum.tile([P, P], F32, name=f"sc0_{layer}", tag="acc0")
                sc1 = ps_accum.tile([P, T], F32, name=f"sc1_{layer}", tag="acc1")
                scores = [sc0, sc1]
                def emit_scores(grp, xr_e, xr_o):
                    ch0 = 4 * grp
                    for q in range(2):  # pair-chunk within group
                        for xr in (xr_e, xr_o):
                            chv = ch0 + 2 * q + (0 if xr is xr_e else 1)
                            nc.tensor.matmul(
                                scores[0],
                                lhsT=xr[:, q, 0:P],
                                rhs=xr[:, q, 0:P],
                                start=(chv == 0),
                                stop=(chv == NCH - 1),
                            )
                            nc.tensor.matmul(
                                scores[1],
                                lhsT=xr[:, q, P:2 * P],
                                rhs=xr[:, q, :],
                                start=(chv == 0),
                                stop=(chv == NCH - 1),
                            )

                pending = None  # (grp, xr_e, xr_o) awaiting scores emission
                for grp in range(NPAIR // 2):  # 16 groups of 2 pair-chunks
                    ch0 = 4 * grp  # first of 4 n-chunks in this group
                    if ch0 % GX == 0:
                        wxg = pwx.tile([P, 2, GX * P], BF16,
                                       name=f"wxg_{layer}_{ch0}", tag="wx")
                        for dk in range(2):
                            nc.sync.dma_start(
                                wxg[:, dk, :],
                                wx_d[P * dk:P * (dk + 1),
                                     P * ch0:P * (ch0 + GX)],
                            )
                    for pc in (2 * grp, 2 * grp + 1):
                        x_pre = ps_work.tile([P, 2 * T], F32,
                                             name=f"xpre_{layer}_{pc}", tag="work")
                        for m in range(2):  # even / odd member chunk
                            ch = 2 * pc + m
                            co = P * (ch % GX)
                            for dk in range(2):
                                nc.tensor.matmul(
                                    x_pre[:, T * m:T * (m + 1)],
                                    lhsT=wxg[:, dk, co:co + P],
                                    rhs=vT[:, dk, :],
                                    start=(dk == 0),
                                    stop=(dk == 1),
                                )
                        nc.scalar.activation(
                            x_sb[:, 2 * pc:2 * pc + 2, :], x_pre, ACT.Relu)
                    # rope over the 2 pair-chunks (even chunks ch0, ch0+2;
                    # odd chunks ch0+1, ch0+3), batched FD=512
                    xe = x_sb[:, ch0:ch0 + 4:2, :]
                    xo = x_sb[:, ch0 + 1:ch0 + 4:2, :]
                    cvw = cs_sb[:, 2 * grp:2 * grp + 2, 0:T]
                    svw = cs_sb[:, 2 * grp:2 * grp + 2, T:2 * T]
                    m_ec = pxr.tile([P, 2, T], BF16, name=f"mec_{layer}_{grp}", tag="xr", bufs=12)
                    m_os = pxr.tile([P, 2, T], BF16, name=f"mos_{layer}_{grp}", tag="xr", bufs=12)
                    m_oc = pxr.tile([P, 2, T], BF16, name=f"moc_{layer}_{grp}", tag="xr", bufs=12)
                    m_es = pxr.tile([P, 2, T], BF16, name=f"mes_{layer}_{grp}", tag="xr", bufs=12)
                    xr_e = pxr.tile([P, 2, T], BF16, name=f"xre_{layer}_{grp}", tag="xr", bufs=12)
                    xr_o = pxr.tile([P, 2, T], BF16, name=f"xro_{layer}_{grp}", tag="xr", bufs=12)
                    nc.vector.tensor_mul(m_ec, xe, cvw)
                    nc.vector.tensor_mul(m_os, xo, svw)
                    nc.vector.tensor_sub(xr_e, m_ec, m_os)
                    nc.vector.tensor_mul(m_oc, xo, cvw)
                    nc.vector.tensor_mul(m_es, xe, svw)
                    nc.vector.tensor_add(xr_o, m_oc, m_es)
                    if pending is not None:
                        emit_scores(*pending)
                    pending = (grp, xr_e, xr_o)
                emit_scores(*pending)

                # ---- softmax (causal, per-head normalized) ----
                # attn packed [128, 384]: cols 0:128 = t-tile0 (s<128),
                # cols 128:384 = t-tile1 (s<256)
                attn = pexp.tile([P, 3 * P], BF16, name=f"attn_{layer}", tag="attn", bufs=2)
                for i, (w, lo) in enumerate(((P, 0), (T, P))):
                    mx = psmall.tile([P, 1], F32, name=f"mx_{i}", tag="lnstat")
                    nc.vector.tensor_reduce(mx, scores[i], axis=AX.X, op=ALU.max)
                    negmx = psmall.tile([P, 1], F32, name=f"negmx_{i}", tag="lnstat")
                    nc.vector.tensor_scalar_mul(negmx, mx, -1.0)
                    ex = pexp.tile([P, w], BF16, name=f"ex_{layer}_{i}", tag="ex", bufs=2)
                    nc.scalar.activation(ex, scores[i], ACT.Exp, bias=negmx)
                    nc.vector.tensor_mul(ex, ex, masks_sb[:, i, 0:w])
                    rs = psmall.tile([P, 1], F32, name=f"rs_{i}", tag="lnstat")
                    nc.vector.tensor_reduce(rs, ex, axis=AX.X, op=ALU.add)
                    rcp = psmall.tile([P, 1], F32, name=f"rcp_{i}", tag="lnstat")
                    nc.vector.reciprocal(rcp, rs)
                    nc.vector.tensor_scalar_mul(attn[:, lo:lo + w], ex, rcp)

                # ---- AllGather attn over the 4-core group; sum heads ----
                attn_bnc = dram.tile([P, 3 * P], BF16,
                                     name=f"attn_bnc_{layer}", tag="attn_in")
                nc.gpsimd.dma_start(attn_bnc[:, 0:P], attn[:, 0:P])
                nc.gpsimd.dma_start(attn_bnc[:, P:3 * P], attn[:, P:3 * P])
                attn_gth = dram.tile([P, 3 * P], BF16, name=f"attn_gth_{layer}",
                                     tag="attn_out")
                nc.gpsimd.collective_compute(
                    "AllReduce", ALU.add, replica_groups=groups,
                    ins=[attn_bnc.opt()], outs=[attn_gth.opt()],
                )
                asum = pexp.tile([P, 3 * P], BF16, name=f"asum_{layer}", tag="asum", bufs=2)
                nc.sync.dma_start(asum, attn_gth[:])

                # ---- transpose summed attn; a = attnT.T @ v; LN(a) ----
                # attnT blocks: b00 = attn[t0, s0].T; b10/b11 = attn[t1, :].T
                attnT = pexp.tile([P, 3 * P], BF16, name=f"attnT_{layer}", tag="attnT", bufs=2)
                for bi, (alo, tlo) in enumerate(((0, 0), (P, P), (2 * P, 2 * P))):
                    tp = ps_work.tile([P, P], BF16, name=f"tpa_{bi}", tag="work")
                    nc.tensor.transpose(tp, asum[:, alo:alo + P], ident_sb)
                    nc.scalar.copy(attnT[:, tlo:tlo + P], tp)
                a_ps = []
                ap_0 = ps_work.tile([P, T], F32, name=f"aps_{layer}_0", tag="work")
                nc.tensor.matmul(ap_0, lhsT=attnT[:, 0:P], rhs=v_bf[:, 0, :],
                                 start=True, stop=True)
                a_ps.append(ap_0)
                ap_1 = ps_work.tile([P, T], F32, name=f"aps_{layer}_1", tag="work")
                for j in range(2):
                    nc.tensor.matmul(
                        ap_1,
                        lhsT=attnT[:, P * (1 + j):P * (2 + j)],
                        rhs=v_bf[:, j, :],
                        start=(j == 0),
                        stop=(j == 1),
                    )
                a_ps.append(ap_1)
                lnA = pbig.tile([P, 2, T], BF16, name=f"lnA_{layer}", tag="lnA")
                _ln_pair(nc, pools, a_ps, lnA)
                lnAT = pbig.tile([P, 2, T], BF16, name=f"lnAT_{layer}", tag="lnAT")
                _transpose4(nc, pools, lnA, lnAT, ident_sb)

                # ---- y phase: y = relu(lnA @ Wy) * x;  yenc = y @ enc ----
                ye0 = ps_accum.tile([P, T], F32, name=f"ye0_{layer}", tag="acc0")
                ye1 = ps_accum.tile([P, T], F32, name=f"ye1_{layer}", tag="acc1")
                yenc = [ye0, ye1]
                def emit_yenc(pc, yt):
                    ch0y = 2 * pc
                    for m in range(2):
                        for i in range(2):
                            nc.tensor.matmul(
                                yenc[i],
                                lhsT=yt[:, T * m + P * i:T * m + P * (i + 1)],
                                rhs=enc_sb[:, ch0y + m, :],
                                start=(ch0y + m == 0),
                                stop=(ch0y + m == NCH - 1),
                            )

                pend_y = None
                for pc in range(NCH // 2):  # two n-chunks at a time
                    ch0y = 2 * pc
                    if ch0y % GX == 0:
                        wyg = pwy.tile([P, 2, GX * P], BF16,
                                       name=f"wyg_{layer}_{ch0y}", tag="wy")
                        for dk in range(2):
                            nc.sync.dma_start(
                                wyg[:, dk, :],
                                wy_d[P * dk:P * (dk + 1), P * ch0y:P * (ch0y + GX)],
                            )
                    y_pre = ps_work.tile([P, 2 * T], F32, name=f"ypre_{layer}_{pc}",
                                         tag="work")
                    for m in range(2):
                        co = P * ((ch0y + m) % GX)
                        for dk in range(2):
                            nc.tensor.matmul(
                                y_pre[:, T * m:T * (m + 1)],
                                lhsT=wyg[:, dk, co:co + P],
                                rhs=lnAT[:, dk, :],
                                start=(dk == 0),
                                stop=(dk == 1),
                            )
                    yr = py.tile([P, 2 * T], BF16, name=f"yr_{layer}_{pc}", tag="y", bufs=8)
                    nc.scalar.activation(yr, y_pre, ACT.Relu)
                    yt = py.tile([P, 2 * T], BF16, name=f"yt_{layer}_{pc}", tag="y", bufs=8)
                    nc.vector.tensor_mul(yt, yr, x_sb[:, ch0y:ch0y + 2, :])
                    if pend_y is not None:
                        emit_yenc(*pend_y)
                    pend_y = (pc, yt)
                emit_yenc(*pend_y)

                # ---- AllGather yenc partials (f32) + sum + LNs + residual ----
                ye_bnc = dram.tile([P, 2 * T], BF16, name=f"ye_bnc_{layer}",
                                   tag="ye_in")
                ye_sb = pexp.tile([P, 2 * T], BF16, name=f"ye_sb_{layer}",
                                  tag="yg", bufs=4)
                for i in range(2):
                    nc.scalar.copy(ye_sb[:, T * i:T * (i + 1)], yenc[i])
                    nc.sync.dma_start(ye_bnc[:, T * i:T * (i + 1)],
                                      ye_sb[:, T * i:T * (i + 1)])
                ye_gth = dram.tile([P, 2 * T], BF16, name=f"ye_gth_{layer}",
                                   tag="ye_out")
                nc.gpsimd.collective_compute(
                    "AllReduce", ALU.add, replica_groups=groups,
                    ins=[ye_bnc.opt()], outs=[ye_gth.opt()],
                )
                ysum = pbig.tile([P, 2, T], BF16, name=f"ysum_{layer}", tag="ysum")
                nc.sync.dma_start(ysum.rearrange("p a t -> p (a t)"), ye_gth[:])
                lnY = pbig.tile([P, 2, T], F32, name=f"lnY_{layer}", tag="lnY")
                _ln_pair(nc, pools, [ysum[:, 0, :], ysum[:, 1, :]], lnY)
                vres = pbig.tile([P, 2, T], F32, name=f"vres_{layer}", tag="vres")
                for i in range(2):
                    nc.vector.tensor_add(vres[:, i, :], v[:, i, :], lnY[:, i, :])
                v = pv.tile([P, 2, T], F32, name=f"v_l{layer + 1}", tag="v")
                _ln_pair(nc, pools, [vres[:, 0, :], vres[:, 1, :]], v)

            # ---- readout: out = v @ ro  (vocab slice) ----
            v_bf = pbig.tile([P, 2, T], BF16, name="vbf_ro", tag="vbf")
            for i in range(2):
                nc.vector.tensor_copy(v_bf[:, i, :], v[:, i, :])
            vT = pbig.tile([P, 2, T], BF16, name="vT_ro", tag="vT")
            _transpose4(nc, pools, v_bf, vT, ident_sb)
            for c in range(NVCH):
                rog = pro.tile([P, 2, VCH], BF16, name=f"rog_{c}", tag="ro")
                for dk in range(2):
                    nc.sync.dma_start(
                        rog[:, dk, :],
                        ro_d[P * dk:P * (dk + 1), VCH * c:VCH * (c + 1)],
                    )
                for i in range(2):
                    lg = ps_work.tile([P, VCH], F32, name=f"lg_{c}_{i}", tag="work")
                    for dk in range(2):
                        nc.tensor.matmul(
                            lg,
                            lhsT=vT[:, dk, P * i:P * (i + 1)],
                            rhs=rog[:, dk, :],
                            start=(dk == 0),
                            stop=(dk == 1),
                        )
                    lg_sb = py.tile([P, VCH], F32, name=f"lg_sb_{c}_{i}",
                                    tag="lgsb", bufs=4)
                    nc.vector.tensor_copy(lg_sb, lg)
                    nc.sync.dma_start(
                        out_d[P * i:P * (i + 1), VCH * c:VCH * (c + 1)], lg_sb
                    )

    nc.compile()
    return nc


# ------------------------- host-side preparation -------------------------

def _pair_perm():
    """perm[new] = old index within a head, de-interleaving rope pairs."""
    perm = np.zeros(n_head, dtype=np.int64)
    for c in range(NPAIR):
        k = np.arange(P) + c * P          # pair indices in this pair-chunk
        perm[(2 * c) * P + np.arange(P)] = 2 * k
        perm[(2 * c + 1) * P + np.arange(P)] = 2 * k + 1
    return perm


def _rope_tables():
    """cs[p, c, 0:T] = cos, cs[p, c, T:2T] = sin, scaled by d**-0.25."""
    inv_freq = 1.0 / (
        ROPE_BASE ** (np.arange(0, n_head, 2, dtype=np.float32) / n_head)
    )  # (4096,) f32, matching reference arithmetic
    t = np.arange(T, dtype=np.float32)
    freqs = t[:, None] * inv_freq[None, :]         # (T, 4096) f32
    cos = np.cos(freqs) * S4                       # (T, 4096)
    sin = np.sin(freqs) * S4
    cs = np.zeros((P, NPAIR, 2 * T), dtype=np.float32)
    for c in range(NPAIR):
        k = c * P + np.arange(P)                   # (128,) pair indices
        cs[:, c, 0:T] = cos[:, k].T
        cs[:, c, T:2 * T] = sin[:, k].T
    return cs.astype(ml_dtypes.bfloat16)


def _masks():
    # [P, 2, T]: tile0 mask in [:, 0, 0:128] (s<=t); tile1 in [:, 1, 0:256]
    m = np.zeros((P, 2, T), dtype=np.float32)
    t = np.arange(P)[:, None]
    m[:, 0, 0:P] = (np.arange(P)[None, :] <= t).astype(np.float32)
    m[:, 1, :] = (np.arange(T)[None, :] <= t + P).astype(np.float32)
    return m.astype(ml_dtypes.bfloat16)


_CACHE = {}


def kernel(idx, wte, encoder, decoder_x, decoder_y, readout):
    if "nc" not in _CACHE:
        _CACHE["nc"] = build_nc()
    nc = _CACHE["nc"]
    in_maps = prepare_in_maps(idx, wte, encoder, decoder_x, decoder_y, readout)
    res = run_bass_kernel_spmd(nc, in_maps, core_ids=list(range(8)))
    return assemble_output([res.results[c]["out"] for c in range(8)])


def assemble_output(outs):
    out = np.empty((B, T, V), dtype=np.float32)
    for c in range(8):
        b, h = c // 4, c % 4
        out[b, :, h * VSLICE:(h + 1) * VSLICE] = outs[c]
    return out


def prepare_in_maps(idx, wte, encoder, decoder_x, decoder_y, readout):
    idx = np.asarray(idx)
    wte = np.ascontiguousarray(np.asarray(wte, dtype=np.float32))
    encoder = np.asarray(encoder, dtype=np.float32)
    decoder_x = np.asarray(decoder_x, dtype=np.float32)
    decoder_y = np.asarray(decoder_y, dtype=np.float32)
    readout = np.asarray(readout, dtype=np.float32)

    perm = _pair_perm()
    cs = _rope_tables()
    masks = _masks()
    ident = np.eye(P, dtype=np.float32).astype(ml_dtypes.bfloat16)
    bf = ml_dtypes.bfloat16

    wx_h = [np.ascontiguousarray(decoder_x[h][:, perm].astype(bf)) for h in range(H)]
    wy_h = [np.ascontiguousarray(decoder_y[h][:, perm].astype(bf)) for h in range(H)]
    enc_h = [
        np.ascontiguousarray(encoder[h * n_head + perm, :].astype(bf))
        for h in range(H)
    ]
    ro_h = [
        np.ascontiguousarray(readout[:, h * VSLICE:(h + 1) * VSLICE].astype(bf))
        for h in range(H)
    ]
    idx_b = [np.ascontiguousarray(idx[b].reshape(2, P).astype(np.int32))
             for b in range(B)]

    in_maps = []
    for c in range(8):
        b, h = c // 4, c % 4
        in_maps.append({
            "wte": wte,
            "idx2": idx_b[b],
            "wx": wx_h[h],
            "wy": wy_h[h],
            "enc": enc_h[h],
            "ro": ro_h[h],
            "cs": cs,
            "masks": masks,
            "ident": ident,
        })

    return in_maps


if __name__ == "__main__":
    nc = build_nc()
    print("built + compiled OK")


# revision 19
# speedup vs baseline: 1.2286x; 1.0234x over previous
"""Trainium2 Bass kernel for the BDH-style weight-tied transformer.

Contract: kernel(**inputs) takes FULL unsharded numpy inputs (idx, wte,
encoder, decoder_x, decoder_y, readout) and returns the FULL (B, T, V)
logits, running the model on 8 NeuronCores via run_bass_kernel_spmd.

Sharding: core c -> (b = c // 4, h = c % 4).  Group {0..3} handles batch 0,
{4..7} batch 1.  Within a group: tensor-parallel over heads with
AllGather + local-sum for (a) the head-summed attention matrix and (b) the
y @ encoder projection.  LayerNorm is scale-invariant, so summing heads
(instead of averaging) is exact.  Readout is vocab-split 4 ways per group.

The neuron axis of each head is permuted host-side so RoPE pair partners
(2k, 2k+1) live at the same partition of sibling 128-chunks ("even" chunk
2c / "odd" chunk 2c+1).  The rotation then needs no cross-partition data
movement.  The 1/sqrt(d) attention scale is folded into the cos/sin tables
(d**-0.25 on each factor of the Gram matrix).
"""

import sys

for _p in ("/opt/trn_rl_repo", "/opt/pypackages"):
    if _p not in sys.path:
        sys.path.append(_p)

import ml_dtypes
import numpy as np

import concourse.bass as bass
import concourse.mybir as mybir
import concourse.tile as tile
from concourse import bacc
from concourse.bass_utils import run_bass_kernel_spmd

F32 = mybir.dt.float32
BF16 = mybir.dt.bfloat16
I32 = mybir.dt.int32
AX = mybir.AxisListType
ALU = mybir.AluOpType
ACT = mybir.ActivationFunctionType

# Model dims (hardcoded per problem spec)
B, T, D, H, N, V = 2, 256, 256, 4, 32768, 32000
n_head = N // H            # 8192 neurons per head (one core's slice)
P = 128
NCH = n_head // P          # 64 chunks of 128 neurons
NPAIR = NCH // 2           # 32 pair-chunks
L_LAYERS = 6
LN_EPS = 1e-5
ROPE_BASE = 10000.0
VSLICE = V // 4            # 8000 vocab columns per core
VCH = 500                  # vocab chunk (PSUM bank holds 512 f32)
NVCH = VSLICE // VCH       # 16
GX = 4                     # n-chunks per streamed weight group
S4 = float(n_head) ** -0.25


def _ln_pair(nc, pools, srcs, out, out_dtype_note=""):
    """LayerNorm over the free dim (D=256) of two [128, 256] f32 tiles.

    srcs: list of 2 APs (SBUF or PSUM, f32).  out: [128, 2, 256] tile.
    """
    psmall = pools["small"]
    for i, src in enumerate(srcs):
        stats = psmall.tile([P, 6], F32, name=f"ln_st{i}", tag="lnstat")
        nc.vector.bn_stats(stats, src)
        aggr = psmall.tile([P, 2], F32, name=f"ln_ag{i}", tag="lnstat")
        nc.vector.bn_aggr(aggr, stats)
        std = psmall.tile([P, 1], F32, name=f"ln_std{i}", tag="lnstat")
        nc.scalar.activation(std, aggr[:, 1:2], ACT.Sqrt,
                             bias=pools["eps"][:, :1])
        rinv = psmall.tile([P, 1], F32, name=f"ln_rinv{i}", tag="lnstat")
        nc.vector.reciprocal(rinv, std)
        nc.vector.tensor_scalar(out[:, i, :], src, aggr[:, 0:1], rinv,
                                op0=ALU.subtract, op1=ALU.mult)


def _transpose4(nc, pools, src, dst, ident):
    """dst[:, k, 128*i:128*(i+1)] = src[:, i, 128*k:128*(k+1)].T  (bf16).

    src, dst: [128, 2, 256] bf16.  Four PE transposes + ACT copies.
    """
    pwork = pools["ps_work"]
    for i in range(2):
        for k in range(2):
            tp = pwork.tile([P, P], BF16, name=f"tp_{i}_{k}", tag="work")
            nc.tensor.transpose(tp, src[:, i, P * k:P * (k + 1)], ident)
            nc.scalar.copy(dst[:, k, P * i:P * (i + 1)], tp)


def build_nc(num_cores=8):
    nc = bacc.Bacc(
        "TRN2", target_bir_lowering=False, debug=False, num_devices=num_cores
    )

    # ---- DRAM I/O (per-core data supplied via in_maps) ----
    wte_d = nc.dram_tensor("wte", [V, D], F32, kind="ExternalInput").ap()
    idx_d = nc.dram_tensor("idx2", [2, P], I32, kind="ExternalInput").ap()
    wx_d = nc.dram_tensor("wx", [D, n_head], BF16, kind="ExternalInput").ap()
    wy_d = nc.dram_tensor("wy", [D, n_head], BF16, kind="ExternalInput").ap()
    enc_d = nc.dram_tensor("enc", [n_head, D], BF16, kind="ExternalInput").ap()
    ro_d = nc.dram_tensor("ro", [D, VSLICE], BF16, kind="ExternalInput").ap()
    cs_d = nc.dram_tensor("cs", [P, NPAIR, 2 * T], BF16, kind="ExternalInput").ap()
    masks_d = nc.dram_tensor("masks", [P, 2, T], BF16, kind="ExternalInput").ap()
    ident_d = nc.dram_tensor("ident", [P, P], BF16, kind="ExternalInput").ap()
    out_d = nc.dram_tensor("out", [T, VSLICE], F32, kind="ExternalOutput").ap()

    groups = [[0, 1, 2, 3], [4, 5, 6, 7]]

    with tile.TileContext(nc) as tc:
        with (
            tc.tile_pool(name="pers", bufs=1) as pers,
            tc.tile_pool(name="pv", bufs=2) as pv,
            tc.tile_pool(name="pbig", bufs=2) as pbig,
            tc.tile_pool(name="pwx", bufs=4) as pwx,
            tc.tile_pool(name="pwy", bufs=4) as pwy,
            tc.tile_pool(name="pro", bufs=3) as pro,
            tc.tile_pool(name="pxr", bufs=8) as pxr,
            tc.tile_pool(name="py", bufs=6) as py,
            tc.tile_pool(name="psmall", bufs=16) as psmall,
            tc.tile_pool(name="pcent", bufs=4) as pcent,
            tc.tile_pool(name="psq", bufs=2) as psq,
            tc.tile_pool(name="pexp", bufs=8) as pexp,
            tc.tile_pool(name="ps_work", bufs=5, space="PSUM") as ps_work,
            tc.tile_pool(name="ps_accum", bufs=1, space="PSUM") as ps_accum,
            tc.tile_pool(name="dram", bufs=2, space="DRAM") as dram,
        ):
            pools = {
                "small": psmall,
                "cent": pcent,
                "sq": psq,
                "ps_work": ps_work,
            }

            # ---- persistent SBUF tensors ----
            eps_sb = pers.tile([P, 1], F32, name="eps_sb", tag="eps")
            nc.vector.memset(eps_sb, LN_EPS)
            pools["eps"] = eps_sb
            cs_sb = pers.tile([P, NPAIR, 2 * T], BF16, name="cs_sb", tag="cs")
            masks_sb = pers.tile([P, 2, T], BF16, name="masks_sb", tag="masks")
            ident_sb = pers.tile([P, P], BF16, name="ident_sb", tag="ident")
            enc_sb = pers.tile([P, NCH, T], BF16, name="enc_sb", tag="enc")
            x_sb = pers.tile([P, NCH, T], BF16, name="x_sb", tag="x")

            for g in range(4):
                nc.sync.dma_start(
                    cs_sb[:, 8 * g:8 * (g + 1), :], cs_d[:, 8 * g:8 * (g + 1), :]
                )
            nc.sync.dma_start(masks_sb[:], masks_d[:])
            nc.sync.dma_start(ident_sb[:], ident_d[:])
            enc_r = enc_d.rearrange("(c p) d -> p c d", p=P)
            for g in range(8):
                nc.sync.dma_start(
                    enc_sb[:, 8 * g:8 * (g + 1), :], enc_r[:, 8 * g:8 * (g + 1), :]
                )

            # ---- warm-up collectives (first CC calls pay ~35us extra);
            # match the real configs (kind/dtype/shape) ----
            warm_sb = pexp.tile([P, 2 * T], BF16, name="warm_sb", tag="warm", bufs=1)
            nc.vector.memset(warm_sb, 0.0)
            wa_in = dram.tile([P, 3 * P], BF16, name="wa_in", tag="attn_in")
            nc.gpsimd.dma_start(wa_in[:], warm_sb[:, 0:3 * P])
            wa_out = dram.tile([P, 3 * P], BF16, name="wa_out", tag="attn_out")
            nc.gpsimd.collective_compute(
                "AllReduce", ALU.add, replica_groups=groups,
                ins=[wa_in.opt()], outs=[wa_out.opt()],
            )
            wy_in = dram.tile([P, 2 * T], BF16, name="wy_in", tag="ye_in")
            nc.gpsimd.dma_start(wy_in[:], warm_sb)
            wy_out = dram.tile([P, 2 * T], BF16, name="wy_out", tag="ye_out")
            nc.gpsimd.collective_compute(
                "AllReduce", ALU.add, replica_groups=groups,
                ins=[wy_in.opt()], outs=[wy_out.opt()],
            )

            # ---- embedding gather + first LN ----
            vraw = pbig.tile([P, 2, T], F32, name="vraw", tag="vraw")
            for i in range(2):
                idx_sb = psmall.tile([P, 1], I32, name=f"idx_sb{i}", tag="idx")
                nc.sync.dma_start(idx_sb, idx_d[i, :].rearrange("(p o) -> p o", o=1))
                nc.gpsimd.indirect_dma_start(
                    out=vraw[:, i, :],
                    out_offset=None,
                    in_=wte_d[:],
                    in_offset=bass.IndirectOffsetOnAxis(ap=idx_sb[:, :1], axis=0),
                )
            v = pv.tile([P, 2, T], F32, name="v_l0", tag="v")
            _ln_pair(nc, pools, [vraw[:, 0, :], vraw[:, 1, :]], v)

            for layer in range(L_LAYERS):
                # ---- v_bf (natural, bf16) and vT (transposed, bf16) ----
                v_bf = pbig.tile([P, 2, T], BF16, name=f"vbf_{layer}", tag="vbf")
                for i in range(2):
                    nc.vector.tensor_copy(v_bf[:, i, :], v[:, i, :])
                vT = pbig.tile([P, 2, T], BF16, name=f"vT_{layer}", tag="vT")
                _transpose4(nc, pools, v_bf, vT, ident_sb)

                # ---- x phase: x = relu(v @ Wx), rope, scores (Gram) ----
                sc0 = ps_accum.tile([P, P], F32, name=f"sc0_{layer}", tag="acc0")
                sc1 = ps_accum.tile([P, T], F32, name=f"sc1_{layer}", tag="acc1")
                scores = [sc0, sc1]
                def emit_scores(grp, xr_e, xr_o):
                    ch0 = 4 * grp
                    for q in range(2):  # pair-chunk within group
                        for xr in (xr_e, xr_o):
                            chv = ch0 + 2 * q + (0 if xr is xr_e else 1)
                            nc.tensor.matmul(
                                scores[0],
                                lhsT=xr[:, q, 0:P],
                                rhs=xr[:, q, 0:P],
                                start=(chv == 0),
                                stop=(chv == NCH - 1),
                            )
                            nc.tensor.matmul(
                                scores[1],
                                lhsT=xr[:, q, P:2 * P],
                                rhs=xr[:, q, :],
                                start=(chv == 0),
                                stop=(chv == NCH - 1),
                            )

                pending = None  # (grp, xr_e, xr_o) awaiting scores emission
                for grp in range(NPAIR // 2):  # 16 groups of 2 pair-chunks
                    ch0 = 4 * grp  # first of 4 n-chunks in this group
                    if ch0 % GX == 0:
                        wxg = pwx.tile([P, 2, GX * P], BF16,
                                       name=f"wxg_{layer}_{ch0}", tag="wx")
                        for dk in range(2):
                            nc.sync.dma_start(
                                wxg[:, dk, :],
                                wx_d[P * dk:P * (dk + 1),
                                     P * ch0:P * (ch0 + GX)],
                            )
                    for pc in (2 * grp, 2 * grp + 1):
                        x_pre = ps_work.tile([P, 2 * T], F32,
                                             name=f"xpre_{layer}_{pc}", tag="work")
                        for m in range(2):  # even / odd member chunk
                            ch = 2 * pc + m
                            co = P * (ch % GX)
                            for dk in range(2):
                                nc.tensor.matmul(
                                    x_pre[:, T * m:T * (m + 1)],
                                    lhsT=wxg[:, dk, co:co + P],
                                    rhs=vT[:, dk, :],
                                    start=(dk == 0),
                                    stop=(dk == 1),
                                )
                        nc.scalar.activation(
                            x_sb[:, 2 * pc:2 * pc + 2, :], x_pre, ACT.Relu)
                    # rope over the 2 pair-chunks (even chunks ch0, ch0+2;
                    # odd chunks ch0+1, ch0+3), batched FD=512
                    xe = x_sb[:, ch0:ch0 + 4:2, :]
                    xo = x_sb[:, ch0 + 1:ch0 + 4:2, :]
                    cvw = cs_sb[:, 2 * grp:2 * grp + 2, 0:T]
                    svw = cs_sb[:, 2 * grp:2 * grp + 2, T:2 * T]
                    m_ec = pxr.tile([P, 2, T], BF16, name=f"mec_{layer}_{grp}", tag="xr", bufs=12)
                    m_os = pxr.tile([P, 2, T], BF16, name=f"mos_{layer}_{grp}", tag="xr", bufs=12)
                    m_oc = pxr.tile([P, 2, T], BF16, name=f"moc_{layer}_{grp}", tag="xr", bufs=12)
                    m_es = pxr.tile([P, 2, T], BF16, name=f"mes_{layer}_{grp}", tag="xr", bufs=12)
                    xr_e = pxr.tile([P, 2, T], BF16, name=f"xre_{layer}_{grp}", tag="xr", bufs=12)
                    xr_o = pxr.tile([P, 2, T], BF16, name=f"xro_{layer}_{grp}", tag="xr", bufs=12)
                    nc.vector.tensor_mul(m_ec, xe, cvw)
                    nc.vector.tensor_mul(m_os, xo, svw)
                    nc.vector.tensor_sub(xr_e, m_ec, m_os)
                    nc.vector.tensor_mul(m_oc, xo, cvw)
                    nc.vector.tensor_mul(m_es, xe, svw)
                    nc.vector.tensor_add(xr_o, m_oc, m_es)
                    if pending is not None:
                        emit_scores(*pending)
                    pending = (grp, xr_e, xr_o)
                emit_scores(*pending)

                # ---- softmax (causal, per-head normalized) ----
                # attn packed [128, 384]: cols 0:128 = t-tile0 (s<128),
                # cols 128:384 = t-tile1 (s<256)
                attn = pexp.tile([P, 3 * P], BF16, name=f"attn_{layer}", tag="attn", bufs=2)
                for i, (w, lo) in enumerate(((P, 0), (T, P))):
                    mx = psmall.tile([P, 1], F32, name=f"mx_{i}", tag="lnstat")
                    nc.vector.tensor_reduce(mx, scores[i], axis=AX.X, op=ALU.max)
                    negmx = psmall.tile([P, 1], F32, name=f"negmx_{i}", tag="lnstat")
                    nc.vector.tensor_scalar_mul(negmx, mx, -1.0)
                    ex = pexp.tile([P, w], BF16, name=f"ex_{layer}_{i}", tag="ex", bufs=2)
                    nc.scalar.activation(ex, scores[i], ACT.Exp, bias=negmx)
                    nc.vector.tensor_mul(ex, ex, masks_sb[:, i, 0:w])
                    rs = psmall.tile([P, 1], F32, name=f"rs_{i}", tag="lnstat")
                    nc.vector.tensor_reduce(rs, ex, axis=AX.X, op=ALU.add)
                    rcp = psmall.tile([P, 1], F32, name=f"rcp_{i}", tag="lnstat")
                    nc.vector.reciprocal(rcp, rs)
                    nc.vector.tensor_scalar_mul(attn[:, lo:lo + w], ex, rcp)

                # ---- AllGather attn over the 4-core group; sum heads ----
                attn_bnc = dram.tile([P, 3 * P], BF16,
                                     name=f"attn_bnc_{layer}", tag="attn_in")
                nc.gpsimd.dma_start(attn_bnc[:, 0:P], attn[:, 0:P])
                nc.gpsimd.dma_start(attn_bnc[:, P:3 * P], attn[:, P:3 * P])
                attn_gth = dram.tile([P, 3 * P], BF16, name=f"attn_gth_{layer}",
                                     tag="attn_out")
                nc.gpsimd.collective_compute(
                    "AllReduce", ALU.add, replica_groups=groups,
                    ins=[attn_bnc.opt()], outs=[attn_gth.opt()],
                )
                asum = pexp.tile([P, 3 * P], BF16, name=f"asum_{layer}", tag="asum", bufs=2)
                nc.sync.dma_start(asum, attn_gth[:])

                # ---- transpose summed attn; a = attnT.T @ v; LN(a) ----
                # attnT blocks: b00 = attn[t0, s0].T; b10/b11 = attn[t1, :].T
                attnT = pexp.tile([P, 3 * P], BF16, name=f"attnT_{layer}", tag="attnT", bufs=2)
                for bi, (alo, tlo) in enumerate(((0, 0), (P, P), (2 * P, 2 * P))):
                    tp = ps_work.tile([P, P], BF16, name=f"tpa_{bi}", tag="work")
                    nc.tensor.transpose(tp, asum[:, alo:alo + P], ident_sb)
                    nc.scalar.copy(attnT[:, tlo:tlo + P], tp)
                a_ps = []
                ap_0 = ps_work.tile([P, T], F32, name=f"aps_{layer}_0", tag="work")
                nc.tensor.matmul(ap_0, lhsT=attnT[:, 0:P], rhs=v_bf[:, 0, :],
                                 start=True, stop=True)
                a_ps.append(ap_0)
                ap_1 = ps_work.tile([P, T], F32, name=f"aps_{layer}_1", tag="work")
                for j in range(2):
                    nc.tensor.matmul(
                        ap_1,
                        lhsT=attnT[:, P * (1 + j):P * (2 + j)],
                        rhs=v_bf[:, j, :],
                        start=(j == 0),
                        stop=(j == 1),
                    )
                a_ps.append(ap_1)
                lnA = pbig.tile([P, 2, T], BF16, name=f"lnA_{layer}", tag="lnA")
                _ln_pair(nc, pools, a_ps, lnA)
                lnAT = pbig.tile([P, 2, T], BF16, name=f"lnAT_{layer}", tag="lnAT")
                _transpose4(nc, pools, lnA, lnAT, ident_sb)

                # ---- y phase: y = relu(lnA @ Wy) * x;  yenc = y @ enc ----
                ye0 = ps_accum.tile([P, T], F32, name=f"ye0_{layer}", tag="acc0")
                ye1 = ps_accum.tile([P, T], F32, name=f"ye1_{layer}", tag="acc1")
                yenc = [ye0, ye1]
                def emit_yenc(pc, yt):
                    ch0y = 2 * pc
                    for m in range(2):
                        for i in range(2):
                            nc.tensor.matmul(
                                yenc[i],
                                lhsT=yt[:, T * m + P * i:T * m + P * (i + 1)],
                                rhs=enc_sb[:, ch0y + m, :],
                                start=(ch0y + m == 0),
                                stop=(ch0y + m == NCH - 1),
                            )

                pend_y = None
                for pc in range(NCH // 2):  # two n-chunks at a time
                    ch0y = 2 * pc
                    if ch0y % GX == 0:
                        wyg = pwy.tile([P, 2, GX * P], BF16,
                                       name=f"wyg_{layer}_{ch0y}", tag="wy")
                        for dk in range(2):
                            nc.sync.dma_start(
                                wyg[:, dk, :],
                                wy_d[P * dk:P * (dk + 1), P * ch0y:P * (ch0y + GX)],
                            )
                    y_pre = ps_work.tile([P, 2 * T], F32, name=f"ypre_{layer}_{pc}",
                                         tag="work")
                    for m in range(2):
                        co = P * ((ch0y + m) % GX)
                        for dk in range(2):
                            nc.tensor.matmul(
                                y_pre[:, T * m:T * (m + 1)],
                                lhsT=wyg[:, dk, co:co + P],
                                rhs=lnAT[:, dk, :],
                                start=(dk == 0),
                                stop=(dk == 1),
                            )
                    yr = py.tile([P, 2 * T], BF16, name=f"yr_{layer}_{pc}", tag="y", bufs=8)
                    nc.scalar.activation(yr, y_pre, ACT.Relu)
                    yt = py.tile([P, 2 * T], BF16, name=f"yt_{layer}_{pc}", tag="y", bufs=8)
                    nc.vector.tensor_mul(yt, yr, x_sb[:, ch0y:ch0y + 2, :])
                    if pend_y is not None:
                        emit_yenc(*pend_y)
                    pend_y = (pc, yt)
                emit_yenc(*pend_y)

                # ---- AllGather yenc partials (f32) + sum + LNs + residual ----
                ye_bnc = dram.tile([P, 2 * T], BF16, name=f"ye_bnc_{layer}",
                                   tag="ye_in")
                ye_sb = pexp.tile([P, 2 * T], BF16, name=f"ye_sb_{layer}",
                                  tag="yg", bufs=4)
                for i in range(2):
                    nc.scalar.copy(ye_sb[:, T * i:T * (i + 1)], yenc[i])
                    nc.sync.dma_start(ye_bnc[:, T * i:T * (i + 1)],
                                      ye_sb[:, T * i:T * (i + 1)])
                ye_gth = dram.tile([P, 2 * T], BF16, name=f"ye_gth_{layer}",
                                   tag="ye_out")
                nc.gpsimd.collective_compute(
                    "AllReduce", ALU.add, replica_groups=groups,
                    ins=[ye_bnc.opt()], outs=[ye_gth.opt()],
                )
                ysum = pbig.tile([P, 2, T], BF16, name=f"ysum_{layer}", tag="ysum")
                nc.sync.dma_start(ysum.rearrange("p a t -> p (a t)"), ye_gth[:])
                lnY = pbig.tile([P, 2, T], F32, name=f"lnY_{layer}", tag="lnY")
                _ln_pair(nc, pools, [ysum[:, 0, :], ysum[:, 1, :]], lnY)
                vres = pbig.tile([P, 2, T], F32, name=f"vres_{layer}", tag="vres")
                for i in range(2):
                    nc.vector.tensor_add(vres[:, i, :], v[:, i, :], lnY[:, i, :])
                v = pv.tile([P, 2, T], F32, name=f"v_l{layer + 1}", tag="v")
                _ln_pair(nc, pools, [vres[:, 0, :], vres[:, 1, :]], v)

            # ---- readout: out = v @ ro  (vocab slice) ----
            v_bf = pbig.tile([P, 2, T], BF16, name="vbf_ro", tag="vbf")
            for i in range(2):
                nc.vector.tensor_copy(v_bf[:, i, :], v[:, i, :])
            vT = pbig.tile([P, 2, T], BF16, name="vT_ro", tag="vT")
            _transpose4(nc, pools, v_bf, vT, ident_sb)
            for c in range(NVCH):
                rog = pro.tile([P, 2, VCH], BF16, name=f"rog_{c}", tag="ro")
                for dk in range(2):
                    nc.sync.dma_start(
                        rog[:, dk, :],
                        ro_d[P * dk:P * (dk + 1), VCH * c:VCH * (c + 1)],
                    )
                for i in range(2):
                    lg = ps_work.tile([P, VCH], F32, name=f"lg_{c}_{i}", tag="work")
                    for dk in range(2):
                        nc.tensor.matmul(
                            lg,
                            lhsT=vT[:, dk, P * i:P * (i + 1)],
                            rhs=rog[:, dk, :],
                            start=(dk == 0),
                            stop=(dk == 1),
                        )
                    lg_sb = py.tile([P, VCH], F32, name=f"lg_sb_{c}_{i}",
                                    tag="lgsb", bufs=4)
                    nc.vector.tensor_copy(lg_sb, lg)
                    nc.sync.dma_start(
                        out_d[P * i:P * (i + 1), VCH * c:VCH * (c + 1)], lg_sb
                    )

    nc.compile()
    return nc


# ------------------------- host-side preparation -------------------------

def _pair_perm():
    """perm[new] = old index within a head, de-interleaving rope pairs."""
    perm = np.zeros(n_head, dtype=np.int64)
    for c in range(NPAIR):
        k = np.arange(P) + c * P          # pair indices in this pair-chunk
        perm[(2 * c) * P + np.arange(P)] = 2 * k
        perm[(2 * c + 1) * P + np.arange(P)] = 2 * k + 1
    return perm


def _rope_tables():
    """cs[p, c, 0:T] = cos, cs[p, c, T:2T] = sin, scaled by d**-0.25."""
    inv_freq = 1.0 / (
        ROPE_BASE ** (np.arange(0, n_head, 2, dtype=np.float32) / n_head)
    )  # (4096,) f32, matching reference arithmetic
    t = np.arange(T, dtype=np.float32)
    freqs = t[:, None] * inv_freq[None, :]         # (T, 4096) f32
    cos = np.cos(freqs) * S4                       # (T, 4096)
    sin = np.sin(freqs) * S4
    cs = np.zeros((P, NPAIR, 2 * T), dtype=np.float32)
    for c in range(NPAIR):
        k = c * P + np.arange(P)                   # (128,) pair indices
        cs[:, c, 0:T] = cos[:, k].T
        cs[:, c, T:2 * T] = sin[:, k].T
    return cs.astype(ml_dtypes.bfloat16)


def _masks():
    # [P, 2, T]: tile0 mask in [:, 0, 0:128] (s<=t); tile1 in [:, 1, 0:256]
    m = np.zeros((P, 2, T), dtype=np.float32)
    t = np.arange(P)[:, None]
    m[:, 0, 0:P] = (np.arange(P)[None, :] <= t).astype(np.float32)
    m[:, 1, :] = (np.arange(T)[None, :] <= t + P).astype(np.float32)
    return m.astype(ml_dtypes.bfloat16)


_CACHE = {}


def kernel(idx, wte, encoder, decoder_x, decoder_y, readout):
    if "nc" not in _CACHE:
        _CACHE["nc"] = build_nc()
    nc = _CACHE["nc"]
    in_maps = prepare_in_maps(idx, wte, encoder, decoder_x, decoder_y, readout)
    res = run_bass_kernel_spmd(nc, in_maps, core_ids=list(range(8)))
    return assemble_output([res.results[c]["out"] for c in range(8)])


def assemble_output(outs):
    out = np.empty((B, T, V), dtype=np.float32)
    for c in range(8):
        b, h = c // 4, c % 4
        out[b, :, h * VSLICE:(h + 1) * VSLICE] = outs[c]
    return out


def prepare_in_maps(idx, wte, encoder, decoder_x, decoder_y, readout):
    idx = np.asarray(idx)
    wte = np.ascontiguousarray(np.asarray(wte, dtype=np.float32))
    encoder = np.asarray(encoder, dtype=np.float32)
    decoder_x = np.asarray(decoder_x, dtype=np.float32)
    decoder_y = np.asarray(decoder_y, dtype=np.float32)
    readout = np.asarray(readout, dtype=np.float32)

    perm = _pair_perm()
    cs = _rope_tables()
    masks = _masks()
    ident = np.eye(P, dtype=np.float32).astype(ml_dtypes.bfloat16)
    bf = ml_dtypes.bfloat16

    wx_h = [np.ascontiguousarray(decoder_x[h][:, perm].astype(bf)) for h in range(H)]
    wy_h = [np.ascontiguousarray(decoder_y[h][:, perm].astype(bf)) for h in range(H)]
    enc_h = [
        np.ascontiguousarray(encoder[h * n_head + perm, :].astype(bf))
        for h in range(H)
    ]
    ro_h = [
        np.ascontiguousarray(readout[:, h * VSLICE:(h + 1) * VSLICE].astype(bf))
        for h in range(H)
    ]
    idx_b = [np.ascontiguousarray(idx[b].reshape(2, P).astype(np.int32))
             for b in range(B)]

    in_maps = []
    for c in range(8):
        b, h = c // 4, c % 4
        in_maps.append({
            "wte": wte,
            "idx2": idx_b[b],
            "wx": wx_h[h],
            "wy": wy_h[h],
            "enc": enc_h[h],
            "ro": ro_h[h],
            "cs": cs,
            "masks": masks,
            "ident": ident,
        })

    return in_maps


if __name__ == "__main__":
    nc = build_nc()
    print("built + compiled OK")


# revision 20
# speedup vs baseline: 1.2364x; 1.0063x over previous
"""Trainium2 Bass kernel for the BDH-style weight-tied transformer.

Contract: kernel(**inputs) takes FULL unsharded numpy inputs (idx, wte,
encoder, decoder_x, decoder_y, readout) and returns the FULL (B, T, V)
logits, running the model on 8 NeuronCores via run_bass_kernel_spmd.

Sharding: core c -> (b = c // 4, h = c % 4).  Group {0..3} handles batch 0,
{4..7} batch 1.  Within a group: tensor-parallel over heads with
AllGather + local-sum for (a) the head-summed attention matrix and (b) the
y @ encoder projection.  LayerNorm is scale-invariant, so summing heads
(instead of averaging) is exact.  Readout is vocab-split 4 ways per group.

The neuron axis of each head is permuted host-side so RoPE pair partners
(2k, 2k+1) live at the same partition of sibling 128-chunks ("even" chunk
2c / "odd" chunk 2c+1).  The rotation then needs no cross-partition data
movement.  The 1/sqrt(d) attention scale is folded into the cos/sin tables
(d**-0.25 on each factor of the Gram matrix).
"""

import sys

for _p in ("/opt/trn_rl_repo", "/opt/pypackages"):
    if _p not in sys.path:
        sys.path.append(_p)

import ml_dtypes
import numpy as np

import concourse.bass as bass
import concourse.mybir as mybir
import concourse.tile as tile
from concourse import bacc
from concourse.bass_utils import run_bass_kernel_spmd

F32 = mybir.dt.float32
BF16 = mybir.dt.bfloat16
I32 = mybir.dt.int32
AX = mybir.AxisListType
ALU = mybir.AluOpType
ACT = mybir.ActivationFunctionType

# Model dims (hardcoded per problem spec)
B, T, D, H, N, V = 2, 256, 256, 4, 32768, 32000
n_head = N // H            # 8192 neurons per head (one core's slice)
P = 128
NCH = n_head // P          # 64 chunks of 128 neurons
NPAIR = NCH // 2           # 32 pair-chunks
L_LAYERS = 6
LN_EPS = 1e-5
ROPE_BASE = 10000.0
VSLICE = V // 4            # 8000 vocab columns per core
VCH = 500                  # vocab chunk (PSUM bank holds 512 f32)
NVCH = VSLICE // VCH       # 16
GX = 8                     # n-chunks per streamed weight group
S4 = float(n_head) ** -0.25


def _ln_pair(nc, pools, srcs, out, out_dtype_note=""):
    """LayerNorm over the free dim (D=256) of two [128, 256] f32 tiles.

    srcs: list of 2 APs (SBUF or PSUM, f32).  out: [128, 2, 256] tile.
    """
    psmall = pools["small"]
    for i, src in enumerate(srcs):
        stats = psmall.tile([P, 6], F32, name=f"ln_st{i}", tag="lnstat")
        nc.vector.bn_stats(stats, src)
        aggr = psmall.tile([P, 2], F32, name=f"ln_ag{i}", tag="lnstat")
        nc.vector.bn_aggr(aggr, stats)
        std = psmall.tile([P, 1], F32, name=f"ln_std{i}", tag="lnstat")
        nc.scalar.activation(std, aggr[:, 1:2], ACT.Sqrt,
                             bias=pools["eps"][:, :1])
        rinv = psmall.tile([P, 1], F32, name=f"ln_rinv{i}", tag="lnstat")
        nc.vector.reciprocal(rinv, std)
        nc.vector.tensor_scalar(out[:, i, :], src, aggr[:, 0:1], rinv,
                                op0=ALU.subtract, op1=ALU.mult)


def _transpose4(nc, pools, src, dst, ident):
    """dst[:, k, 128*i:128*(i+1)] = src[:, i, 128*k:128*(k+1)].T  (bf16).

    src, dst: [128, 2, 256] bf16.  Four PE transposes + ACT copies.
    """
    pwork = pools["ps_work"]
    for i in range(2):
        for k in range(2):
            tp = pwork.tile([P, P], BF16, name=f"tp_{i}_{k}", tag="work")
            nc.tensor.transpose(tp, src[:, i, P * k:P * (k + 1)], ident)
            nc.scalar.copy(dst[:, k, P * i:P * (i + 1)], tp)


def build_nc(num_cores=8):
    nc = bacc.Bacc(
        "TRN2", target_bir_lowering=False, debug=False, num_devices=num_cores
    )

    # ---- DRAM I/O (per-core data supplied via in_maps) ----
    wte_d = nc.dram_tensor("wte", [V, D], F32, kind="ExternalInput").ap()
    idx_d = nc.dram_tensor("idx2", [2, P], I32, kind="ExternalInput").ap()
    wx_d = nc.dram_tensor("wx", [D, n_head], BF16, kind="ExternalInput").ap()
    wy_d = nc.dram_tensor("wy", [D, n_head], BF16, kind="ExternalInput").ap()
    enc_d = nc.dram_tensor("enc", [n_head, D], BF16, kind="ExternalInput").ap()
    ro_d = nc.dram_tensor("ro", [D, VSLICE], BF16, kind="ExternalInput").ap()
    cs_d = nc.dram_tensor("cs", [P, NPAIR, 2 * T], BF16, kind="ExternalInput").ap()
    masks_d = nc.dram_tensor("masks", [P, 2, T], BF16, kind="ExternalInput").ap()
    ident_d = nc.dram_tensor("ident", [P, P], BF16, kind="ExternalInput").ap()
    out_d = nc.dram_tensor("out", [T, VSLICE], F32, kind="ExternalOutput").ap()

    groups = [[0, 1, 2, 3], [4, 5, 6, 7]]

    with tile.TileContext(nc) as tc:
        with (
            tc.tile_pool(name="pers", bufs=1) as pers,
            tc.tile_pool(name="pv", bufs=2) as pv,
            tc.tile_pool(name="pbig", bufs=2) as pbig,
            tc.tile_pool(name="pwx", bufs=3) as pwx,
            tc.tile_pool(name="pwy", bufs=3) as pwy,
            tc.tile_pool(name="pro", bufs=3) as pro,
            tc.tile_pool(name="pxr", bufs=8) as pxr,
            tc.tile_pool(name="py", bufs=6) as py,
            tc.tile_pool(name="psmall", bufs=16) as psmall,
            tc.tile_pool(name="pcent", bufs=4) as pcent,
            tc.tile_pool(name="psq", bufs=2) as psq,
            tc.tile_pool(name="pexp", bufs=8) as pexp,
            tc.tile_pool(name="ps_work", bufs=5, space="PSUM") as ps_work,
            tc.tile_pool(name="ps_accum", bufs=1, space="PSUM") as ps_accum,
            tc.tile_pool(name="dram", bufs=2, space="DRAM") as dram,
        ):
            pools = {
                "small": psmall,
                "cent": pcent,
                "sq": psq,
                "ps_work": ps_work,
            }

            # ---- persistent SBUF tensors ----
            eps_sb = pers.tile([P, 1], F32, name="eps_sb", tag="eps")
            nc.vector.memset(eps_sb, LN_EPS)
            pools["eps"] = eps_sb
            cs_sb = pers.tile([P, NPAIR, 2 * T], BF16, name="cs_sb", tag="cs")
            masks_sb = pers.tile([P, 2, T], BF16, name="masks_sb", tag="masks")
            ident_sb = pers.tile([P, P], BF16, name="ident_sb", tag="ident")
            enc_sb = pers.tile([P, NCH, T], BF16, name="enc_sb", tag="enc")
            x_sb = pers.tile([P, NCH, T], BF16, name="x_sb", tag="x")

            for g in range(4):
                nc.sync.dma_start(
                    cs_sb[:, 8 * g:8 * (g + 1), :], cs_d[:, 8 * g:8 * (g + 1), :]
                )
            nc.sync.dma_start(masks_sb[:], masks_d[:])
            nc.sync.dma_start(ident_sb[:], ident_d[:])
            enc_r = enc_d.rearrange("(c p) d -> p c d", p=P)
            for g in range(8):
                nc.sync.dma_start(
                    enc_sb[:, 8 * g:8 * (g + 1), :], enc_r[:, 8 * g:8 * (g + 1), :]
                )

            # ---- warm-up collectives (first CC calls pay ~35us extra);
            # match the real configs (kind/dtype/shape) ----
            warm_sb = pexp.tile([P, 2 * T], BF16, name="warm_sb", tag="warm", bufs=1)
            nc.vector.memset(warm_sb, 0.0)
            wa_in = dram.tile([P, 3 * P], BF16, name="wa_in", tag="attn_in")
            nc.gpsimd.dma_start(wa_in[:], warm_sb[:, 0:3 * P])
            wa_out = dram.tile([P, 3 * P], BF16, name="wa_out", tag="attn_out")
            nc.gpsimd.collective_compute(
                "AllReduce", ALU.add, replica_groups=groups,
                ins=[wa_in.opt()], outs=[wa_out.opt()],
            )
            wy_in = dram.tile([P, 2 * T], BF16, name="wy_in", tag="ye_in")
            nc.gpsimd.dma_start(wy_in[:], warm_sb)
            wy_out = dram.tile([P, 2 * T], BF16, name="wy_out", tag="ye_out")
            nc.gpsimd.collective_compute(
                "AllReduce", ALU.add, replica_groups=groups,
                ins=[wy_in.opt()], outs=[wy_out.opt()],
            )

            # ---- embedding gather + first LN ----
            vraw = pbig.tile([P, 2, T], F32, name="vraw", tag="vraw")
            for i in range(2):
                idx_sb = psmall.tile([P, 1], I32, name=f"idx_sb{i}", tag="idx")
                nc.sync.dma_start(idx_sb, idx_d[i, :].rearrange("(p o) -> p o", o=1))
                nc.gpsimd.indirect_dma_start(
                    out=vraw[:, i, :],
                    out_offset=None,
                    in_=wte_d[:],
                    in_offset=bass.IndirectOffsetOnAxis(ap=idx_sb[:, :1], axis=0),
                )
            v = pv.tile([P, 2, T], F32, name="v_l0", tag="v")
            _ln_pair(nc, pools, [vraw[:, 0, :], vraw[:, 1, :]], v)

            for layer in range(L_LAYERS):
                # ---- v_bf (natural, bf16) and vT (transposed, bf16) ----
                v_bf = pbig.tile([P, 2, T], BF16, name=f"vbf_{layer}", tag="vbf")
                for i in range(2):
                    nc.vector.tensor_copy(v_bf[:, i, :], v[:, i, :])
                vT = pbig.tile([P, 2, T], BF16, name=f"vT_{layer}", tag="vT")
                _transpose4(nc, pools, v_bf, vT, ident_sb)

                # ---- x phase: x = relu(v @ Wx), rope, scores (Gram) ----
                sc0 = ps_accum.tile([P, P], F32, name=f"sc0_{layer}", tag="acc0")
                sc1 = ps_accum.tile([P, T], F32, name=f"sc1_{layer}", tag="acc1")
                scores = [sc0, sc1]
                def emit_scores(grp, xr_e, xr_o):
                    ch0 = 4 * grp
                    for q in range(2):  # pair-chunk within group
                        for xr in (xr_e, xr_o):
                            chv = ch0 + 2 * q + (0 if xr is xr_e else 1)
                            nc.tensor.matmul(
                                scores[0],
                                lhsT=xr[:, q, 0:P],
                                rhs=xr[:, q, 0:P],
                                start=(chv == 0),
                                stop=(chv == NCH - 1),
                            )
                            nc.tensor.matmul(
                                scores[1],
                                lhsT=xr[:, q, P:2 * P],
                                rhs=xr[:, q, :],
                                start=(chv == 0),
                                stop=(chv == NCH - 1),
                            )

                pending = None  # (grp, xr_e, xr_o) awaiting scores emission
                for grp in range(NPAIR // 2):  # 16 groups of 2 pair-chunks
                    ch0 = 4 * grp  # first of 4 n-chunks in this group
                    if ch0 % GX == 0:
                        wxg = pwx.tile([P, 2, GX * P], BF16,
                                       name=f"wxg_{layer}_{ch0}", tag="wx")
                        for dk in range(2):
                            nc.sync.dma_start(
                                wxg[:, dk, :],
                                wx_d[P * dk:P * (dk + 1),
                                     P * ch0:P * (ch0 + GX)],
                            )
                    for pc in (2 * grp, 2 * grp + 1):
                        x_pre = ps_work.tile([P, 2 * T], F32,
                                             name=f"xpre_{layer}_{pc}", tag="work")
                        for m in range(2):  # even / odd member chunk
                            ch = 2 * pc + m
                            co = P * (ch % GX)
                            for dk in range(2):
                                nc.tensor.matmul(
                                    x_pre[:, T * m:T * (m + 1)],
                                    lhsT=wxg[:, dk, co:co + P],
                                    rhs=vT[:, dk, :],
                                    start=(dk == 0),
                                    stop=(dk == 1),
                                )
                        nc.scalar.activation(
                            x_sb[:, 2 * pc:2 * pc + 2, :], x_pre, ACT.Relu)
                    # rope over the 2 pair-chunks (even chunks ch0, ch0+2;
                    # odd chunks ch0+1, ch0+3), batched FD=512
                    xe = x_sb[:, ch0:ch0 + 4:2, :]
                    xo = x_sb[:, ch0 + 1:ch0 + 4:2, :]
                    cvw = cs_sb[:, 2 * grp:2 * grp + 2, 0:T]
                    svw = cs_sb[:, 2 * grp:2 * grp + 2, T:2 * T]
                    m_ec = pxr.tile([P, 2, T], BF16, name=f"mec_{layer}_{grp}", tag="xr", bufs=12)
                    m_os = pxr.tile([P, 2, T], BF16, name=f"mos_{layer}_{grp}", tag="xr", bufs=12)
                    m_oc = pxr.tile([P, 2, T], BF16, name=f"moc_{layer}_{grp}", tag="xr", bufs=12)
                    m_es = pxr.tile([P, 2, T], BF16, name=f"mes_{layer}_{grp}", tag="xr", bufs=12)
                    xr_e = pxr.tile([P, 2, T], BF16, name=f"xre_{layer}_{grp}", tag="xr", bufs=12)
                    xr_o = pxr.tile([P, 2, T], BF16, name=f"xro_{layer}_{grp}", tag="xr", bufs=12)
                    nc.vector.tensor_mul(m_ec, xe, cvw)
                    nc.vector.tensor_mul(m_os, xo, svw)
                    nc.vector.tensor_sub(xr_e, m_ec, m_os)
                    nc.vector.tensor_mul(m_oc, xo, cvw)
                    nc.vector.tensor_mul(m_es, xe, svw)
                    nc.vector.tensor_add(xr_o, m_oc, m_es)
                    if pending is not None:
                        emit_scores(*pending)
                    pending = (grp, xr_e, xr_o)
                emit_scores(*pending)

                # ---- softmax (causal, per-head normalized) ----
                # attn packed [128, 384]: cols 0:128 = t-tile0 (s<128),
                # cols 128:384 = t-tile1 (s<256)
                attn = pexp.tile([P, 3 * P], BF16, name=f"attn_{layer}", tag="attn", bufs=2)
                for i, (w, lo) in enumerate(((P, 0), (T, P))):
                    mx = psmall.tile([P, 1], F32, name=f"mx_{i}", tag="lnstat")
                    nc.vector.tensor_reduce(mx, scores[i], axis=AX.X, op=ALU.max)
                    negmx = psmall.tile([P, 1], F32, name=f"negmx_{i}", tag="lnstat")
                    nc.vector.tensor_scalar_mul(negmx, mx, -1.0)
                    ex = pexp.tile([P, w], BF16, name=f"ex_{layer}_{i}", tag="ex", bufs=2)
                    nc.scalar.activation(ex, scores[i], ACT.Exp, bias=negmx)
                    nc.vector.tensor_mul(ex, ex, masks_sb[:, i, 0:w])
                    rs = psmall.tile([P, 1], F32, name=f"rs_{i}", tag="lnstat")
                    nc.vector.tensor_reduce(rs, ex, axis=AX.X, op=ALU.add)
                    rcp = psmall.tile([P, 1], F32, name=f"rcp_{i}", tag="lnstat")
                    nc.vector.reciprocal(rcp, rs)
                    nc.vector.tensor_scalar_mul(attn[:, lo:lo + w], ex, rcp)

                # ---- AllGather attn over the 4-core group; sum heads ----
                attn_bnc = dram.tile([P, 3 * P], BF16,
                                     name=f"attn_bnc_{layer}", tag="attn_in")
                nc.gpsimd.dma_start(attn_bnc[:, 0:P], attn[:, 0:P])
                nc.gpsimd.dma_start(attn_bnc[:, P:3 * P], attn[:, P:3 * P])
                attn_gth = dram.tile([P, 3 * P], BF16, name=f"attn_gth_{layer}",
                                     tag="attn_out")
                nc.gpsimd.collective_compute(
                    "AllReduce", ALU.add, replica_groups=groups,
                    ins=[attn_bnc.opt()], outs=[attn_gth.opt()],
                )
                asum = pexp.tile([P, 3 * P], BF16, name=f"asum_{layer}", tag="asum", bufs=2)
                nc.sync.dma_start(asum, attn_gth[:])

                # ---- transpose summed attn; a = attnT.T @ v; LN(a) ----
                # attnT blocks: b00 = attn[t0, s0].T; b10/b11 = attn[t1, :].T
                attnT = pexp.tile([P, 3 * P], BF16, name=f"attnT_{layer}", tag="attnT", bufs=2)
                for bi, (alo, tlo) in enumerate(((0, 0), (P, P), (2 * P, 2 * P))):
                    tp = ps_work.tile([P, P], BF16, name=f"tpa_{bi}", tag="work")
                    nc.tensor.transpose(tp, asum[:, alo:alo + P], ident_sb)
                    nc.scalar.copy(attnT[:, tlo:tlo + P], tp)
                a_ps = []
                ap_0 = ps_work.tile([P, T], F32, name=f"aps_{layer}_0", tag="work")
                nc.tensor.matmul(ap_0, lhsT=attnT[:, 0:P], rhs=v_bf[:, 0, :],
                                 start=True, stop=True)
                a_ps.append(ap_0)
                ap_1 = ps_work.tile([P, T], F32, name=f"aps_{layer}_1", tag="work")
                for j in range(2):
                    nc.tensor.matmul(
                        ap_1,
                        lhsT=attnT[:, P * (1 + j):P * (2 + j)],
                        rhs=v_bf[:, j, :],
                        start=(j == 0),
                        stop=(j == 1),
                    )
                a_ps.append(ap_1)
                lnA = pbig.tile([P, 2, T], BF16, name=f"lnA_{layer}", tag="lnA")
                _ln_pair(nc, pools, a_ps, lnA)
                lnAT = pbig.tile([P, 2, T], BF16, name=f"lnAT_{layer}", tag="lnAT")
                _transpose4(nc, pools, lnA, lnAT, ident_sb)

                # ---- y phase: y = relu(lnA @ Wy) * x;  yenc = y @ enc ----
                ye0 = ps_accum.tile([P, T], F32, name=f"ye0_{layer}", tag="acc0")
                ye1 = ps_accum.tile([P, T], F32, name=f"ye1_{layer}", tag="acc1")
                yenc = [ye0, ye1]
                def emit_yenc(pc, yt):
                    ch0y = 2 * pc
                    for m in range(2):
                        for i in range(2):
                            nc.tensor.matmul(
                                yenc[i],
                                lhsT=yt[:, T * m + P * i:T * m + P * (i + 1)],
                                rhs=enc_sb[:, ch0y + m, :],
                                start=(ch0y + m == 0),
                                stop=(ch0y + m == NCH - 1),
                            )

                pend_y = None
                for pc in range(NCH // 2):  # two n-chunks at a time
                    ch0y = 2 * pc
                    if ch0y % GX == 0:
                        wyg = pwy.tile([P, 2, GX * P], BF16,
                                       name=f"wyg_{layer}_{ch0y}", tag="wy")
                        for dk in range(2):
                            nc.sync.dma_start(
                                wyg[:, dk, :],
                                wy_d[P * dk:P * (dk + 1), P * ch0y:P * (ch0y + GX)],
                            )
                    y_pre = ps_work.tile([P, 2 * T], F32, name=f"ypre_{layer}_{pc}",
                                         tag="work")
                    for m in range(2):
                        co = P * ((ch0y + m) % GX)
                        for dk in range(2):
                            nc.tensor.matmul(
                                y_pre[:, T * m:T * (m + 1)],
                                lhsT=wyg[:, dk, co:co + P],
                                rhs=lnAT[:, dk, :],
                                start=(dk == 0),
                                stop=(dk == 1),
                            )
                    yr = py.tile([P, 2 * T], BF16, name=f"yr_{layer}_{pc}", tag="y", bufs=8)
                    nc.scalar.activation(yr, y_pre, ACT.Relu)
                    yt = py.tile([P, 2 * T], BF16, name=f"yt_{layer}_{pc}", tag="y", bufs=8)
                    nc.vector.tensor_mul(yt, yr, x_sb[:, ch0y:ch0y + 2, :])
                    if pend_y is not None:
                        emit_yenc(*pend_y)
                    pend_y = (pc, yt)
                emit_yenc(*pend_y)

                # ---- AllGather yenc partials (f32) + sum + LNs + residual ----
                ye_bnc = dram.tile([P, 2 * T], BF16, name=f"ye_bnc_{layer}",
                                   tag="ye_in")
                ye_sb = pexp.tile([P, 2 * T], BF16, name=f"ye_sb_{layer}",
                                  tag="yg", bufs=4)
                for i in range(2):
                    nc.scalar.copy(ye_sb[:, T * i:T * (i + 1)], yenc[i])
                    nc.sync.dma_start(ye_bnc[:, T * i:T * (i + 1)],
                                      ye_sb[:, T * i:T * (i + 1)])
                ye_gth = dram.tile([P, 2 * T], BF16, name=f"ye_gth_{layer}",
                                   tag="ye_out")
                nc.gpsimd.collective_compute(
                    "AllReduce", ALU.add, replica_groups=groups,
                    ins=[ye_bnc.opt()], outs=[ye_gth.opt()],
                )
                ysum = pbig.tile([P, 2, T], BF16, name=f"ysum_{layer}", tag="ysum")
                nc.sync.dma_start(ysum.rearrange("p a t -> p (a t)"), ye_gth[:])
                lnY = pbig.tile([P, 2, T], F32, name=f"lnY_{layer}", tag="lnY")
                _ln_pair(nc, pools, [ysum[:, 0, :], ysum[:, 1, :]], lnY)
                vres = pbig.tile([P, 2, T], F32, name=f"vres_{layer}", tag="vres")
                for i in range(2):
                    nc.vector.tensor_add(vres[:, i, :], v[:, i, :], lnY[:, i, :])
                v = pv.tile([P, 2, T], F32, name=f"v_l{layer + 1}", tag="v")
                _ln_pair(nc, pools, [vres[:, 0, :], vres[:, 1, :]], v)

            # ---- readout: out = v @ ro  (vocab slice) ----
            v_bf = pbig.tile([P, 2, T], BF16, name="vbf_ro", tag="vbf")
            for i in range(2):
                nc.vector.tensor_copy(v_bf[:, i, :], v[:, i, :])
            vT = pbig.tile([P, 2, T], BF16, name="vT_ro", tag="vT")
            _transpose4(nc, pools, v_bf, vT, ident_sb)
            for c in range(NVCH):
                rog = pro.tile([P, 2, VCH], BF16, name=f"rog_{c}", tag="ro")
                for dk in range(2):
                    nc.sync.dma_start(
                        rog[:, dk, :],
                        ro_d[P * dk:P * (dk + 1), VCH * c:VCH * (c + 1)],
                    )
                for i in range(2):
                    lg = ps_work.tile([P, VCH], F32, name=f"lg_{c}_{i}", tag="work")
                    for dk in range(2):
                        nc.tensor.matmul(
                            lg,
                            lhsT=vT[:, dk, P * i:P * (i + 1)],
                            rhs=rog[:, dk, :],
                            start=(dk == 0),
                            stop=(dk == 1),
                        )
                    lg_sb = py.tile([P, VCH], F32, name=f"lg_sb_{c}_{i}",
                                    tag="lgsb", bufs=4)
                    nc.vector.tensor_copy(lg_sb, lg)
                    nc.sync.dma_start(
                        out_d[P * i:P * (i + 1), VCH * c:VCH * (c + 1)], lg_sb
                    )

    nc.compile()
    return nc


# ------------------------- host-side preparation -------------------------

def _pair_perm():
    """perm[new] = old index within a head, de-interleaving rope pairs."""
    perm = np.zeros(n_head, dtype=np.int64)
    for c in range(NPAIR):
        k = np.arange(P) + c * P          # pair indices in this pair-chunk
        perm[(2 * c) * P + np.arange(P)] = 2 * k
        perm[(2 * c + 1) * P + np.arange(P)] = 2 * k + 1
    return perm


def _rope_tables():
    """cs[p, c, 0:T] = cos, cs[p, c, T:2T] = sin, scaled by d**-0.25."""
    inv_freq = 1.0 / (
        ROPE_BASE ** (np.arange(0, n_head, 2, dtype=np.float32) / n_head)
    )  # (4096,) f32, matching reference arithmetic
    t = np.arange(T, dtype=np.float32)
    freqs = t[:, None] * inv_freq[None, :]         # (T, 4096) f32
    cos = np.cos(freqs) * S4                       # (T, 4096)
    sin = np.sin(freqs) * S4
    cs = np.zeros((P, NPAIR, 2 * T), dtype=np.float32)
    for c in range(NPAIR):
        k = c * P + np.arange(P)                   # (128,) pair indices
        cs[:, c, 0:T] = cos[:, k].T
        cs[:, c, T:2 * T] = sin[:, k].T
    return cs.astype(ml_dtypes.bfloat16)


def _masks():
    # [P, 2, T]: tile0 mask in [:, 0, 0:128] (s<=t); tile1 in [:, 1, 0:256]
    m = np.zeros((P, 2, T), dtype=np.float32)
    t = np.arange(P)[:, None]
    m[:, 0, 0:P] = (np.arange(P)[None, :] <= t).astype(np.float32)
    m[:, 1, :] = (np.arange(T)[None, :] <= t + P).astype(np.float32)
    return m.astype(ml_dtypes.bfloat16)


_CACHE = {}


def kernel(idx, wte, encoder, decoder_x, decoder_y, readout):
    if "nc" not in _CACHE:
        _CACHE["nc"] = build_nc()
    nc = _CACHE["nc"]
    in_maps = prepare_in_maps(idx, wte, encoder, decoder_x, decoder_y, readout)
    res = run_bass_kernel_spmd(nc, in_maps, core_ids=list(range(8)))
    return assemble_output([res.results[c]["out"] for c in range(8)])


def assemble_output(outs):
    out = np.empty((B, T, V), dtype=np.float32)
    for c in range(8):
        b, h = c // 4, c % 4
        out[b, :, h * VSLICE:(h + 1) * VSLICE] = outs[c]
    return out


def prepare_in_maps(idx, wte, encoder, decoder_x, decoder_y, readout):
    idx = np.asarray(idx)
    wte = np.ascontiguousarray(np.asarray(wte, dtype=np.float32))
    encoder = np.asarray(encoder, dtype=np.float32)
    decoder_x = np.asarray(decoder_x, dtype=np.float32)
    decoder_y = np.asarray(decoder_y, dtype=np.float32)
    readout = np.asarray(readout, dtype=np.float32)

    perm = _pair_perm()
    cs = _rope_tables()
    masks = _masks()
    ident = np.eye(P, dtype=np.float32).astype(ml_dtypes.bfloat16)
    bf = ml_dtypes.bfloat16

    wx_h = [np.ascontiguousarray(decoder_x[h][:, perm].astype(bf)) for h in range(H)]
    wy_h = [np.ascontiguousarray(decoder_y[h][:, perm].astype(bf)) for h in range(H)]
    enc_h = [
        np.ascontiguousarray(encoder[h * n_head + perm, :].astype(bf))
        for h in range(H)
    ]
    ro_h = [
        np.ascontiguousarray(readout[:, h * VSLICE:(h + 1) * VSLICE].astype(bf))
        for h in range(H)
    ]
    idx_b = [np.ascontiguousarray(idx[b].reshape(2, P).astype(np.int32))
             for b in range(B)]

    in_maps = []
    for c in range(8):
        b, h = c // 4, c % 4
        in_maps.append({
            "wte": wte,
            "idx2": idx_b[b],
            "wx": wx_h[h],
            "wy": wy_h[h],
            "enc": enc_h[h],
            "ro": ro_h[h],
            "cs": cs,
            "masks": masks,
            "ident": ident,
        })

    return in_maps


if __name__ == "__main__":
    nc = build_nc()
    print("built + compiled OK")
